# revision 20
# baseline (speedup 1.0000x reference)
"""Trainium2 Bass kernel for nn_Attention_46454366273781 (sparse_attention).

Reference computation (T=2048, B=32, N=1024, H=8, K=128, K2=16):
    X = einsum('tbn,hkn->bthk', hyp, Wmh) + bmh          # per-head projections
    m = X.mean(axis=1)                                   # mean over time
    g = tanh(X @ W.T + bW) * tanh(m @ Wm.T + bWm)[:,None]
    s = g @ Wh + bWh ; a = softmax(s, axis=time)
    c = einsum('bth,bthk->bhk', a, X) ; out = c.reshape(B, H*K)

Key algebra: X itself is never needed on device.
  * scoring:  X @ W.T + bW  =  hyp @ WS.T + bSp   with WS = W @ Wmh (per head)
  * gate:     m @ Wm.T + bWm = mean_t(hyp) @ WSm.T + bSm,  WSm = Wm @ Wmh
  * output:   softmax weights sum to 1, so
              c_bh = (a_bh^T hyp_b) @ Wmh_h^T + bmh_h  (the v-trick)
This turns the 137-GFLOP projection into a rank-128 scoring matmul plus two
passes over hyp (one N-major for scoring/mean, one T-major for the weighted
sum), making the kernel DMA-bound.  Sharding: data-parallel over batch B
across 8 cores (4 batches/core).  bWh cancels inside the softmax.
"""

import numpy as np
import ml_dtypes

T, B, N, H = 2048, 32, 1024, 8
K, K2 = 128, 16          # per-head dim, attention hidden per head
NCORES = 8
BL = B // NCORES         # batches per core
TC = 512                 # time chunk for scoring matmul free dim
NCH = N // 128           # contraction chunks over N
TCH = T // TC            # time chunks (scoring)
T128 = T // 128          # 128-sized time chunks

_cache = {}


def _build_nc():
    import concourse.mybir as mybir
    import concourse.tile as tile
    from concourse import bacc
    from concourse.masks import make_identity

    bf16 = mybir.dt.bfloat16
    f32 = mybir.dt.float32
    AF = mybir.ActivationFunctionType
    AX = mybir.AxisListType
    OP = mybir.AluOpType

    nc = bacc.Bacc("TRN2")
    hypT = nc.dram_tensor("hypT", (BL, N, T), bf16, kind="ExternalInput")
    hypN = nc.dram_tensor("hypN", (BL, T, N), bf16, kind="ExternalInput")
    wmhT_d = nc.dram_tensor("wmhT", (128, H, NCH, K), bf16, kind="ExternalInput")
    WST_d = nc.dram_tensor("WST", (128, NCH, 128), bf16, kind="ExternalInput")
    WSmT_d = nc.dram_tensor("WSmT", (128, NCH, 128), bf16, kind="ExternalInput")
    whD_d = nc.dram_tensor("whD", (K, H), bf16, kind="ExternalInput")
    bSp_d = nc.dram_tensor("bSp", (128, 1), f32, kind="ExternalInput")
    bSm_d = nc.dram_tensor("bSm", (128, 1), f32, kind="ExternalInput")
    bmhT_d = nc.dram_tensor("bmhT", (K, H), f32, kind="ExternalInput")
    out_d = nc.dram_tensor("out", (BL, K, H), f32, kind="ExternalOutput")

    with tile.TileContext(nc) as tc, \
         tc.tile_pool(name="wpool", bufs=1) as wpool, \
         tc.tile_pool(name="hypTp", bufs=2 * NCH) as hypTp, \
         tc.tile_pool(name="hypNp", bufs=2 * T128) as hypNp, \
         tc.tile_pool(name="gp", bufs=3) as gp, \
         tc.tile_pool(name="aTp", bufs=2 * T128) as aTp, \
         tc.tile_pool(name="seqp", bufs=2) as seqp, \
         tc.tile_pool(name="smallp", bufs=2) as smallp, \
         tc.tile_pool(name="psA", bufs=2, space="PSUM") as psA, \
         tc.tile_pool(name="psV", bufs=2, space="PSUM") as psV, \
         tc.tile_pool(name="psS", bufs=4, space="PSUM") as psS:

        # ---- constants / weights (loaded once; wmhT last, c-phase only) ----
        WST = wpool.tile([128, NCH, 128], bf16)
        nc.sync.dma_start(out=WST, in_=WST_d[:])
        WSmT = wpool.tile([128, NCH, 128], bf16)
        nc.sync.dma_start(out=WSmT, in_=WSmT_d[:])
        whD = wpool.tile([K, H], bf16)
        nc.sync.dma_start(out=whD, in_=whD_d[:])
        bSp = wpool.tile([128, 1], f32)
        nc.sync.dma_start(out=bSp, in_=bSp_d[:])
        bSm = wpool.tile([128, 1], f32)
        nc.sync.dma_start(out=bSm, in_=bSm_d[:])
        bmhT = wpool.tile([K, H], f32)
        nc.sync.dma_start(out=bmhT, in_=bmhT_d[:])
        ident = wpool.tile([128, 128], bf16)
        make_identity(nc, ident)
        wmhT = wpool.tile([128, H, NCH, K], bf16)
        dump = wpool.tile([128, T], bf16)   # write-only sink for mean pass

        for bl in range(BL):
            # ---- load hyp in both layouts ----
            hT = [hypTp.tile([128, T], bf16, tag="hT", name=f"hT_{bl}_{i}")
                  for i in range(NCH)]
            for half in range(2):
                hsl = slice(half * (T // 2), (half + 1) * (T // 2))
                for n in range(NCH):
                    nc.sync.dma_start(out=hT[n][:, hsl],
                                      in_=hypT[bl, n * 128:(n + 1) * 128, hsl])
            hN = [hypNp.tile([128, N], bf16, tag="hN", name=f"hN_{bl}_{i}")
                  for i in range(T128)]
            for t in range(T128):
                nc.sync.dma_start(out=hN[t],
                                  in_=hypN[bl, t * 128:(t + 1) * 128, :])
            if bl == 0:
                # c-phase weights: load behind bl0's data, ahead of first use
                nc.sync.dma_start(out=wmhT, in_=wmhT_d[:])

            # ---- gate: mw = tanh(WSm @ mean_t(hyp)^T + bSm), packed [hq,1] ----
            hmT = smallp.tile([128, NCH], f32, tag="hmT", name=f"hmT_{bl}")
            for n in range(NCH):
                if n % 2 == 0:
                    nc.scalar.activation(out=dump, in_=hT[n], func=AF.Copy,
                                         accum_out=hmT[:, n:n + 1])
                else:
                    nc.vector.reduce_sum(out=hmT[:, n:n + 1], in_=hT[n],
                                         axis=AX.X)
            hmT_bf = smallp.tile([128, NCH], bf16, tag="hmT_bf",
                                 name=f"hmT_bf_{bl}")
            nc.scalar.activation(out=hmT_bf, in_=hmT, func=AF.Copy,
                                 scale=1.0 / T)
            ps_mwp = psS.tile([128, 1], f32, tag="psS", name=f"ps_mwp_{bl}")
            for n in range(NCH):
                nc.tensor.matmul(ps_mwp, lhsT=WSmT[:, n, :],
                                 rhs=hmT_bf[:, n:n + 1],
                                 start=(n == 0), stop=(n == NCH - 1))
            mwP = smallp.tile([128, 1], f32, tag="mwP", name=f"mwP_{bl}")
            nc.scalar.activation(out=mwP, in_=ps_mwp, func=AF.Tanh, bias=bSm)

            # ---- scoring: s = whD^T (tanh(WS hyp^T + bSp) * mwP); softmax ----
            s_exp = seqp.tile([8, T], bf16, tag="s_exp", name=f"s_exp_{bl}")
            ssum_parts = smallp.tile([8, TCH], f32, tag="ssum_parts",
                                     name=f"ssum_parts_{bl}")
            for tci in range(TCH):
                tsl = slice(tci * TC, (tci + 1) * TC)
                ps = psA.tile([128, TC], f32, tag="psA", name=f"psA_{bl}_{tci}")
                for n in range(NCH):
                    nc.tensor.matmul(ps, lhsT=WST[:, n, :],
                                     rhs=hT[n][:, tsl],
                                     start=(n == 0), stop=(n == NCH - 1))
                g1 = gp.tile([128, TC], f32, tag="g1", name=f"g1_{bl}_{tci}")
                nc.scalar.activation(out=g1, in_=ps, func=AF.Tanh, bias=bSp)
                g2 = gp.tile([128, TC], bf16, tag="g2", name=f"g2_{bl}_{tci}")
                nc.vector.tensor_scalar_mul(g2, g1, mwP)
                ps_s = psS.tile([8, TC], f32, tag="psS", name=f"ps_s_{bl}_{tci}")
                nc.tensor.matmul(ps_s, lhsT=whD, rhs=g2, start=True, stop=True)
                nc.scalar.activation(out=s_exp[:, tsl], in_=ps_s, func=AF.Exp,
                                     accum_out=ssum_parts[:, tci:tci + 1])
            ssum = smallp.tile([8, 1], f32, tag="ssum", name=f"ssum_{bl}")
            nc.vector.reduce_sum(out=ssum, in_=ssum_parts, axis=AX.X)
            sinv = smallp.tile([8, 1], f32, tag="sinv", name=f"sinv_{bl}")
            nc.vector.reciprocal(sinv, ssum)

            # ---- v = a^T hyp ; c^T = v @ Wmh^T + bmh ----
            aT = []
            for t in range(T128):
                ps_aT = psS.tile([128, 8], bf16, tag="psS",
                                 name=f"ps_aT_{bl}_{t}")
                nc.tensor.transpose(ps_aT, s_exp[:, t * 128:(t + 1) * 128],
                                    ident[:8, :8])
                aTt = aTp.tile([128, 8], bf16, tag="aT", name=f"aT_{bl}_{t}")
                nc.scalar.copy(aTt, ps_aT)
                aT.append(aTt)
            v_sb = smallp.tile([8, N], bf16, tag="v_sb", name=f"v_sb_{bl}")
            ps_v = [psV.tile([8, 512], f32, tag="psV", name=f"ps_v_{bl}_{i}")
                    for i in range(2)]
            for t in range(T128):
                for nh in range(2):
                    nc.tensor.matmul(ps_v[nh], lhsT=aT[t],
                                     rhs=hN[t][:, nh * 512:(nh + 1) * 512],
                                     start=(t == 0), stop=(t == T128 - 1),
                                     skip_group_check=True)
            for nh in range(2):
                nc.scalar.activation(out=v_sb[:, nh * 512:(nh + 1) * 512],
                                     in_=ps_v[nh], func=AF.Copy, scale=sinv)
            vT = smallp.tile([128, NCH, 8], bf16, tag="vT", name=f"vT_{bl}")
            for n in range(NCH):
                ps_vT = psS.tile([128, 8], bf16, tag="psS",
                                 name=f"ps_vT_{bl}_{n}")
                nc.tensor.transpose(ps_vT, v_sb[:, n * 128:(n + 1) * 128],
                                    ident[:8, :8])
                nc.scalar.copy(vT[:, n, :], ps_vT)
            ps_cT = psS.tile([128, H], f32, tag="psS", name=f"ps_cT_{bl}")
            for h in range(H):
                for n in range(NCH):
                    nc.tensor.matmul(ps_cT[:, h:h + 1], lhsT=wmhT[:, h, n, :],
                                     rhs=vT[:, n, h:h + 1],
                                     start=(n == 0), stop=(n == NCH - 1),
                                     skip_group_check=True)
            c2 = smallp.tile([128, H], f32, tag="c2", name=f"c2_{bl}")
            nc.vector.tensor_tensor(out=c2, in0=ps_cT, in1=bmhT, op=OP.add)
            nc.sync.dma_start(out=out_d[bl], in_=c2)

    nc.compile()
    return nc


def _prep_inputs(hyp, Wmh, bmh, W, bW, Wm, bWm, Wh, bWh):
    """Host-side sharding + layout prep (numpy only)."""
    bf = ml_dtypes.bfloat16
    hyp = np.asarray(hyp, np.float32)
    Wmh = np.asarray(Wmh, np.float32)
    bmh = np.asarray(bmh, np.float32)
    W = np.asarray(W, np.float32)
    bW = np.asarray(bW, np.float32)
    Wm = np.asarray(Wm, np.float32)
    bWm = np.asarray(bWm, np.float32)
    Wh = np.asarray(Wh, np.float32)

    hyp_b = np.ascontiguousarray(hyp.transpose(1, 0, 2))          # (B, T, N)
    hypN_all = hyp_b.astype(bf)
    hypT_all = np.ascontiguousarray(hyp_b.transpose(0, 2, 1)).astype(bf)

    # Wmh (H, K, N) -> (128, H, NCH, K): [p, h, n, k] = Wmh[h, k, n*128+p]
    wmhT = np.ascontiguousarray(
        Wmh.transpose(2, 0, 1).reshape(NCH, 128, H, K)
        .transpose(1, 2, 0, 3)).astype(bf)

    # fused scoring weights: WS[h*16+q, n] = sum_k W[q,k] Wmh[h,k,n]
    WS = np.einsum('qk,hkn->hqn', W, Wmh).reshape(128, N)
    WST = np.ascontiguousarray(
        WS.T.reshape(NCH, 128, 128).transpose(1, 0, 2)).astype(bf)
    bSp = (np.einsum('qk,hk->hq', W, bmh).reshape(128)
           + np.tile(bW, H)).astype(np.float32).reshape(128, 1)

    WSm = np.einsum('qk,hkn->hqn', Wm, Wmh).reshape(128, N)
    WSmT = np.ascontiguousarray(
        WSm.T.reshape(NCH, 128, 128).transpose(1, 0, 2)).astype(bf)
    bSm = (np.einsum('qk,hk->hq', Wm, bmh).reshape(128)
           + np.tile(bWm, H)).astype(np.float32).reshape(128, 1)

    whD = np.zeros((K, H), dtype=np.float32)
    for h in range(H):
        whD[h * K2:(h + 1) * K2, h] = Wh
    whD = whD.astype(bf)
    bmhT = np.ascontiguousarray(bmh.T)                            # (K, H)

    in_maps = []
    for c in range(NCORES):
        sl = slice(c * BL, (c + 1) * BL)
        in_maps.append({
            "hypT": np.ascontiguousarray(hypT_all[sl]),
            "hypN": np.ascontiguousarray(hypN_all[sl]),
            "wmhT": wmhT, "WST": WST, "WSmT": WSmT, "whD": whD,
            "bSp": bSp, "bSm": bSm, "bmhT": bmhT,
        })
    return in_maps


def kernel(hyp, Wmh, bmh, W, bW, Wm, bWm, Wh, bWh,
           dan_hidden_size=None, attention_hidden_size=None,
           multihead_size=None, **_):
    from concourse.bass_utils import run_bass_kernel_spmd

    in_maps = _prep_inputs(hyp, Wmh, bmh, W, bW, Wm, bWm, Wh, bWh)
    if "nc" not in _cache:
        _cache["nc"] = _build_nc()
    res = run_bass_kernel_spmd(_cache["nc"], in_maps, core_ids=list(range(NCORES)))
    # out is (BL, K, H) per core -> (B, H, K) -> (B, N)
    out = np.concatenate([r["out"].transpose(0, 2, 1).reshape(BL, N)
                          for r in res.results], axis=0)
    return out.astype(np.float32)


# revision 21
# speedup vs baseline: 1.0069x; 1.0069x over previous
"""Trainium2 Bass kernel for nn_Attention_46454366273781 (sparse_attention).

Reference computation (T=2048, B=32, N=1024, H=8, K=128, K2=16):
    X = einsum('tbn,hkn->bthk', hyp, Wmh) + bmh          # per-head projections
    m = X.mean(axis=1)                                   # mean over time
    g = tanh(X @ W.T + bW) * tanh(m @ Wm.T + bWm)[:,None]
    s = g @ Wh + bWh ; a = softmax(s, axis=time)
    c = einsum('bth,bthk->bhk', a, X) ; out = c.reshape(B, H*K)

Key algebra: X itself is never needed on device.
  * scoring:  X @ W.T + bW  =  hyp @ WS.T + bSp   with WS = W @ Wmh (per head)
  * gate:     m @ Wm.T + bWm = mean_t(hyp) @ WSm.T + bSm,  WSm = Wm @ Wmh
  * output:   softmax weights sum to 1, so
              c_bh = (a_bh^T hyp_b) @ Wmh_h^T + bmh_h  (the v-trick)
This turns the 137-GFLOP projection into a rank-128 scoring matmul plus two
passes over hyp (one N-major for scoring/mean, one T-major for the weighted
sum), making the kernel DMA-bound.  Sharding: data-parallel over batch B
across 8 cores (4 batches/core).  bWh cancels inside the softmax.
"""

import numpy as np
import ml_dtypes

T, B, N, H = 2048, 32, 1024, 8
K, K2 = 128, 16          # per-head dim, attention hidden per head
NCORES = 8
BL = B // NCORES         # batches per core
TC = 512                 # time chunk for scoring matmul free dim
NCH = N // 128           # contraction chunks over N
TCH = T // TC            # time chunks (scoring)
T128 = T // 128          # 128-sized time chunks

_cache = {}


def _build_nc():
    import concourse.mybir as mybir
    import concourse.tile as tile
    from concourse import bacc
    from concourse.masks import make_identity

    bf16 = mybir.dt.bfloat16
    f32 = mybir.dt.float32
    AF = mybir.ActivationFunctionType
    AX = mybir.AxisListType
    OP = mybir.AluOpType

    nc = bacc.Bacc("TRN2")
    hypT = nc.dram_tensor("hypT", (BL, N, T), bf16, kind="ExternalInput")
    hypN = nc.dram_tensor("hypN", (BL, T, N), bf16, kind="ExternalInput")
    wmhT_d = nc.dram_tensor("wmhT", (128, H, NCH, K), bf16, kind="ExternalInput")
    WST_d = nc.dram_tensor("WST", (128, NCH, 128), bf16, kind="ExternalInput")
    WSmT_d = nc.dram_tensor("WSmT", (128, NCH, 128), bf16, kind="ExternalInput")
    whD_d = nc.dram_tensor("whD", (K, H), bf16, kind="ExternalInput")
    bSp_d = nc.dram_tensor("bSp", (128, 1), f32, kind="ExternalInput")
    bSm_d = nc.dram_tensor("bSm", (128, 1), f32, kind="ExternalInput")
    bmhT_d = nc.dram_tensor("bmhT", (K, H), f32, kind="ExternalInput")
    out_d = nc.dram_tensor("out", (BL, K, H), f32, kind="ExternalOutput")

    with tile.TileContext(nc) as tc, \
         tc.tile_pool(name="wpool", bufs=1) as wpool, \
         tc.tile_pool(name="hypTp", bufs=2 * NCH) as hypTp, \
         tc.tile_pool(name="hypNp", bufs=2 * T128) as hypNp, \
         tc.tile_pool(name="gp", bufs=3) as gp, \
         tc.tile_pool(name="aTp", bufs=2 * T128) as aTp, \
         tc.tile_pool(name="seqp", bufs=2) as seqp, \
         tc.tile_pool(name="smallp", bufs=2) as smallp, \
         tc.tile_pool(name="psA", bufs=2, space="PSUM") as psA, \
         tc.tile_pool(name="psV", bufs=2, space="PSUM") as psV, \
         tc.tile_pool(name="psS", bufs=4, space="PSUM") as psS:

        # ---- constants / weights (loaded once; wmhT last, c-phase only) ----
        WST = wpool.tile([128, NCH, 128], bf16)
        nc.sync.dma_start(out=WST, in_=WST_d[:])
        WSmT = wpool.tile([128, NCH, 128], bf16)
        nc.sync.dma_start(out=WSmT, in_=WSmT_d[:])
        whD = wpool.tile([K, H], bf16)
        nc.sync.dma_start(out=whD, in_=whD_d[:])
        bSp = wpool.tile([128, 1], f32)
        nc.sync.dma_start(out=bSp, in_=bSp_d[:])
        bSm = wpool.tile([128, 1], f32)
        nc.sync.dma_start(out=bSm, in_=bSm_d[:])
        bmhT = wpool.tile([K, H], f32)
        nc.sync.dma_start(out=bmhT, in_=bmhT_d[:])
        ident = wpool.tile([128, 128], bf16)
        make_identity(nc, ident)
        wmhT = wpool.tile([128, H, NCH, K], bf16)
        dump = wpool.tile([128, T], bf16)   # write-only sink for mean pass

        for bl in range(BL):
            # ---- load hyp in both layouts ----
            hT = [hypTp.tile([128, T], bf16, tag="hT", name=f"hT_{bl}_{i}")
                  for i in range(NCH)]
            for half in range(2):
                hsl = slice(half * (T // 2), (half + 1) * (T // 2))
                for n in range(NCH):
                    nc.sync.dma_start(out=hT[n][:, hsl],
                                      in_=hypT[bl, n * 128:(n + 1) * 128, hsl])
            hN = [hypNp.tile([128, N], bf16, tag="hN", name=f"hN_{bl}_{i}")
                  for i in range(T128)]
            for t in range(T128):
                nc.sync.dma_start(out=hN[t],
                                  in_=hypN[bl, t * 128:(t + 1) * 128, :])
            if bl == 0:
                # c-phase weights: load behind bl0's data, ahead of first use
                nc.sync.dma_start(out=wmhT, in_=wmhT_d[:])

            # ---- gate: mw = tanh(WSm @ mean_t(hyp)^T + bSm), packed [hq,1] ----
            hmT = smallp.tile([128, NCH], f32, tag="hmT", name=f"hmT_{bl}")
            for n in range(NCH):
                if n % 2 == 0:
                    nc.scalar.activation(out=dump, in_=hT[n], func=AF.Copy,
                                         accum_out=hmT[:, n:n + 1])
                else:
                    nc.vector.reduce_sum(out=hmT[:, n:n + 1], in_=hT[n],
                                         axis=AX.X)
            hmT_bf = smallp.tile([128, NCH], bf16, tag="hmT_bf",
                                 name=f"hmT_bf_{bl}")
            nc.scalar.activation(out=hmT_bf, in_=hmT, func=AF.Copy,
                                 scale=1.0 / T)
            ps_mwp = psS.tile([128, 1], f32, tag="psS", name=f"ps_mwp_{bl}")
            for n in range(NCH):
                nc.tensor.matmul(ps_mwp, lhsT=WSmT[:, n, :],
                                 rhs=hmT_bf[:, n:n + 1],
                                 start=(n == 0), stop=(n == NCH - 1))
            mwP = smallp.tile([128, 1], f32, tag="mwP", name=f"mwP_{bl}")
            nc.scalar.activation(out=mwP, in_=ps_mwp, func=AF.Tanh, bias=bSm)

            # ---- scoring: s = whD^T (tanh(WS hyp^T + bSp) * mwP); softmax ----
            s_exp = seqp.tile([8, T], bf16, tag="s_exp", name=f"s_exp_{bl}")
            ssum_parts = smallp.tile([8, TCH], f32, tag="ssum_parts",
                                     name=f"ssum_parts_{bl}")
            for tci in range(TCH):
                tsl = slice(tci * TC, (tci + 1) * TC)
                ps = psA.tile([128, TC], f32, tag="psA", name=f"psA_{bl}_{tci}")
                for n in range(NCH):
                    nc.tensor.matmul(ps, lhsT=WST[:, n, :],
                                     rhs=hT[n][:, tsl],
                                     start=(n == 0), stop=(n == NCH - 1))
                g1 = gp.tile([128, TC], f32, tag="g1", name=f"g1_{bl}_{tci}")
                nc.scalar.activation(out=g1, in_=ps, func=AF.Tanh, bias=bSp)
                g2 = gp.tile([128, TC], bf16, tag="g2", name=f"g2_{bl}_{tci}")
                nc.vector.tensor_scalar_mul(g2, g1, mwP)
                ps_s = psS.tile([8, TC], f32, tag="psS", name=f"ps_s_{bl}_{tci}")
                nc.tensor.matmul(ps_s, lhsT=whD, rhs=g2, start=True, stop=True)
                nc.scalar.activation(out=s_exp[:, tsl], in_=ps_s, func=AF.Exp,
                                     accum_out=ssum_parts[:, tci:tci + 1])
            ssum = smallp.tile([8, 1], f32, tag="ssum", name=f"ssum_{bl}")
            nc.vector.reduce_sum(out=ssum, in_=ssum_parts, axis=AX.X)
            sinv = smallp.tile([8, 1], f32, tag="sinv", name=f"sinv_{bl}")
            nc.vector.reciprocal(sinv, ssum)

            # ---- v = a^T hyp ; c^T = v @ Wmh^T + bmh ----
            aT = []
            for t in range(T128):
                ps_aT = psS.tile([128, 8], bf16, tag="psS",
                                 name=f"ps_aT_{bl}_{t}")
                nc.tensor.transpose(ps_aT, s_exp[:, t * 128:(t + 1) * 128],
                                    ident[:8, :8])
                aTt = aTp.tile([128, 8], bf16, tag="aT", name=f"aT_{bl}_{t}")
                nc.scalar.copy(aTt, ps_aT)
                aT.append(aTt)
            v_sb = smallp.tile([8, N], bf16, tag="v_sb", name=f"v_sb_{bl}")
            for nh in range(2):
                ps_v = psV.tile([8, 512], f32, tag="psV",
                                name=f"ps_v_{bl}_{nh}")
                for t in range(T128):
                    nc.tensor.matmul(ps_v, lhsT=aT[t],
                                     rhs=hN[t][:, nh * 512:(nh + 1) * 512],
                                     start=(t == 0), stop=(t == T128 - 1))
                nc.scalar.activation(out=v_sb[:, nh * 512:(nh + 1) * 512],
                                     in_=ps_v, func=AF.Copy, scale=sinv)
            vT = smallp.tile([128, NCH, 8], bf16, tag="vT", name=f"vT_{bl}")
            for n in range(NCH):
                ps_vT = psS.tile([128, 8], bf16, tag="psS",
                                 name=f"ps_vT_{bl}_{n}")
                nc.tensor.transpose(ps_vT, v_sb[:, n * 128:(n + 1) * 128],
                                    ident[:8, :8])
                nc.scalar.copy(vT[:, n, :], ps_vT)
            ps_cT = psS.tile([128, H], f32, tag="psS", name=f"ps_cT_{bl}")
            for h in range(H):
                for n in range(NCH):
                    nc.tensor.matmul(ps_cT[:, h:h + 1], lhsT=wmhT[:, h, n, :],
                                     rhs=vT[:, n, h:h + 1],
                                     start=(n == 0), stop=(n == NCH - 1),
                                     skip_group_check=True)
            c2 = smallp.tile([128, H], f32, tag="c2", name=f"c2_{bl}")
            nc.vector.tensor_tensor(out=c2, in0=ps_cT, in1=bmhT, op=OP.add)
            nc.sync.dma_start(out=out_d[bl], in_=c2)

    nc.compile()
    return nc


def _prep_inputs(hyp, Wmh, bmh, W, bW, Wm, bWm, Wh, bWh):
    """Host-side sharding + layout prep (numpy only)."""
    bf = ml_dtypes.bfloat16
    hyp = np.asarray(hyp, np.float32)
    Wmh = np.asarray(Wmh, np.float32)
    bmh = np.asarray(bmh, np.float32)
    W = np.asarray(W, np.float32)
    bW = np.asarray(bW, np.float32)
    Wm = np.asarray(Wm, np.float32)
    bWm = np.asarray(bWm, np.float32)
    Wh = np.asarray(Wh, np.float32)

    hyp_b = np.ascontiguousarray(hyp.transpose(1, 0, 2))          # (B, T, N)
    hypN_all = hyp_b.astype(bf)
    hypT_all = np.ascontiguousarray(hyp_b.transpose(0, 2, 1)).astype(bf)

    # Wmh (H, K, N) -> (128, H, NCH, K): [p, h, n, k] = Wmh[h, k, n*128+p]
    wmhT = np.ascontiguousarray(
        Wmh.transpose(2, 0, 1).reshape(NCH, 128, H, K)
        .transpose(1, 2, 0, 3)).astype(bf)

    # fused scoring weights: WS[h*16+q, n] = sum_k W[q,k] Wmh[h,k,n]
    WS = np.einsum('qk,hkn->hqn', W, Wmh).reshape(128, N)
    WST = np.ascontiguousarray(
        WS.T.reshape(NCH, 128, 128).transpose(1, 0, 2)).astype(bf)
    bSp = (np.einsum('qk,hk->hq', W, bmh).reshape(128)
           + np.tile(bW, H)).astype(np.float32).reshape(128, 1)

    WSm = np.einsum('qk,hkn->hqn', Wm, Wmh).reshape(128, N)
    WSmT = np.ascontiguousarray(
        WSm.T.reshape(NCH, 128, 128).transpose(1, 0, 2)).astype(bf)
    bSm = (np.einsum('qk,hk->hq', Wm, bmh).reshape(128)
           + np.tile(bWm, H)).astype(np.float32).reshape(128, 1)

    whD = np.zeros((K, H), dtype=np.float32)
    for h in range(H):
        whD[h * K2:(h + 1) * K2, h] = Wh
    whD = whD.astype(bf)
    bmhT = np.ascontiguousarray(bmh.T)                            # (K, H)

    in_maps = []
    for c in range(NCORES):
        sl = slice(c * BL, (c + 1) * BL)
        in_maps.append({
            "hypT": np.ascontiguousarray(hypT_all[sl]),
            "hypN": np.ascontiguousarray(hypN_all[sl]),
            "wmhT": wmhT, "WST": WST, "WSmT": WSmT, "whD": whD,
            "bSp": bSp, "bSm": bSm, "bmhT": bmhT,
        })
    return in_maps


def kernel(hyp, Wmh, bmh, W, bW, Wm, bWm, Wh, bWh,
           dan_hidden_size=None, attention_hidden_size=None,
           multihead_size=None, **_):
    from concourse.bass_utils import run_bass_kernel_spmd

    in_maps = _prep_inputs(hyp, Wmh, bmh, W, bW, Wm, bWm, Wh, bWh)
    if "nc" not in _cache:
        _cache["nc"] = _build_nc()
    res = run_bass_kernel_spmd(_cache["nc"], in_maps, core_ids=list(range(NCORES)))
    # out is (BL, K, H) per core -> (B, H, K) -> (B, N)
    out = np.concatenate([r["out"].transpose(0, 2, 1).reshape(BL, N)
                          for r in res.results], axis=0)
    return out.astype(np.float32)


# revision 22
# speedup vs baseline: 1.0125x; 1.0055x over previous
"""Trainium2 Bass kernel for nn_Attention_46454366273781 (sparse_attention).

Reference computation (T=2048, B=32, N=1024, H=8, K=128, K2=16):
    X = einsum('tbn,hkn->bthk', hyp, Wmh) + bmh          # per-head projections
    m = X.mean(axis=1)                                   # mean over time
    g = tanh(X @ W.T + bW) * tanh(m @ Wm.T + bWm)[:,None]
    s = g @ Wh + bWh ; a = softmax(s, axis=time)
    c = einsum('bth,bthk->bhk', a, X) ; out = c.reshape(B, H*K)

Key algebra: X itself is never needed on device.
  * scoring:  X @ W.T + bW  =  hyp @ WS.T + bSp   with WS = W @ Wmh (per head)
  * gate:     m @ Wm.T + bWm = mean_t(hyp) @ WSm.T + bSm,  WSm = Wm @ Wmh
  * output:   softmax weights sum to 1, so
              c_bh = (a_bh^T hyp_b) @ Wmh_h^T + bmh_h  (the v-trick)
This turns the 137-GFLOP projection into a rank-128 scoring matmul plus two
passes over hyp (one N-major for scoring/mean, one T-major for the weighted
sum), making the kernel DMA-bound.  Sharding: data-parallel over batch B
across 8 cores (4 batches/core).  bWh cancels inside the softmax.
"""

import numpy as np
import ml_dtypes

T, B, N, H = 2048, 32, 1024, 8
K, K2 = 128, 16          # per-head dim, attention hidden per head
NCORES = 8
BL = B // NCORES         # batches per core
TC = 512                 # time chunk for scoring matmul free dim
NCH = N // 128           # contraction chunks over N
TCH = T // TC            # time chunks (scoring)
T128 = T // 128          # 128-sized time chunks

_cache = {}


def _build_nc():
    import concourse.mybir as mybir
    import concourse.tile as tile
    from concourse import bacc
    from concourse.masks import make_identity

    bf16 = mybir.dt.bfloat16
    f32 = mybir.dt.float32
    AF = mybir.ActivationFunctionType
    AX = mybir.AxisListType
    OP = mybir.AluOpType

    nc = bacc.Bacc("TRN2")
    hypT = nc.dram_tensor("hypT", (BL, N, T), bf16, kind="ExternalInput")
    hypN = nc.dram_tensor("hypN", (BL, T, N), bf16, kind="ExternalInput")
    wmhT_d = nc.dram_tensor("wmhT", (128, H, NCH, K), bf16, kind="ExternalInput")
    WST_d = nc.dram_tensor("WST", (128, NCH, 128), bf16, kind="ExternalInput")
    WSmT_d = nc.dram_tensor("WSmT", (128, NCH, 128), bf16, kind="ExternalInput")
    whD_d = nc.dram_tensor("whD", (K, H), bf16, kind="ExternalInput")
    bSp_d = nc.dram_tensor("bSp", (128, 1), f32, kind="ExternalInput")
    bSm_d = nc.dram_tensor("bSm", (128, 1), f32, kind="ExternalInput")
    bmhT_d = nc.dram_tensor("bmhT", (K, H), f32, kind="ExternalInput")
    out_d = nc.dram_tensor("out", (BL, K, H), f32, kind="ExternalOutput")

    with tile.TileContext(nc) as tc, \
         tc.tile_pool(name="wpool", bufs=1) as wpool, \
         tc.tile_pool(name="hypTp", bufs=2 * NCH) as hypTp, \
         tc.tile_pool(name="hypNp", bufs=2 * T128) as hypNp, \
         tc.tile_pool(name="gp", bufs=3) as gp, \
         tc.tile_pool(name="aTp", bufs=2 * T128) as aTp, \
         tc.tile_pool(name="seqp", bufs=2) as seqp, \
         tc.tile_pool(name="smallp", bufs=2) as smallp, \
         tc.tile_pool(name="psA", bufs=2, space="PSUM") as psA, \
         tc.tile_pool(name="psV", bufs=2, space="PSUM") as psV, \
         tc.tile_pool(name="psS", bufs=4, space="PSUM") as psS:

        # ---- constants / weights (loaded once; wmhT last, c-phase only) ----
        WST = wpool.tile([128, NCH, 128], bf16)
        nc.sync.dma_start(out=WST, in_=WST_d[:])
        WSmT = wpool.tile([128, NCH, 128], bf16)
        nc.sync.dma_start(out=WSmT, in_=WSmT_d[:])
        whD = wpool.tile([K, H], bf16)
        nc.sync.dma_start(out=whD, in_=whD_d[:])
        bSp = wpool.tile([128, 1], f32)
        nc.sync.dma_start(out=bSp, in_=bSp_d[:])
        bSm = wpool.tile([128, 1], f32)
        nc.sync.dma_start(out=bSm, in_=bSm_d[:])
        bmhT = wpool.tile([K, H], f32)
        nc.sync.dma_start(out=bmhT, in_=bmhT_d[:])
        ident = wpool.tile([128, 128], bf16)
        make_identity(nc, ident)
        wmhT = wpool.tile([128, H, NCH, K], bf16)
        nc.sync.dma_start(out=wmhT, in_=wmhT_d[:])
        dump = wpool.tile([128, T], bf16)   # write-only sink for mean pass

        for bl in range(BL):
            # ---- load hyp in both layouts ----
            hT = [hypTp.tile([128, T], bf16, tag="hT", name=f"hT_{bl}_{i}")
                  for i in range(NCH)]
            for half in range(2):
                hsl = slice(half * (T // 2), (half + 1) * (T // 2))
                for n in range(NCH):
                    nc.sync.dma_start(out=hT[n][:, hsl],
                                      in_=hypT[bl, n * 128:(n + 1) * 128, hsl])
            hN = [hypNp.tile([128, N], bf16, tag="hN", name=f"hN_{bl}_{i}")
                  for i in range(T128)]
            for t in range(T128):
                nc.sync.dma_start(out=hN[t],
                                  in_=hypN[bl, t * 128:(t + 1) * 128, :])

            # ---- gate: mw = tanh(WSm @ mean_t(hyp)^T + bSm), packed [hq,1] ----
            hmT = smallp.tile([128, NCH], f32, tag="hmT", name=f"hmT_{bl}")
            for n in range(NCH):
                if n % 2 == 0:
                    nc.scalar.activation(out=dump, in_=hT[n], func=AF.Copy,
                                         accum_out=hmT[:, n:n + 1])
                else:
                    nc.vector.reduce_sum(out=hmT[:, n:n + 1], in_=hT[n],
                                         axis=AX.X)
            hmT_bf = smallp.tile([128, NCH], bf16, tag="hmT_bf",
                                 name=f"hmT_bf_{bl}")
            nc.scalar.activation(out=hmT_bf, in_=hmT, func=AF.Copy,
                                 scale=1.0 / T)
            ps_mwp = psS.tile([128, 1], f32, tag="psS", name=f"ps_mwp_{bl}")
            for n in range(NCH):
                nc.tensor.matmul(ps_mwp, lhsT=WSmT[:, n, :],
                                 rhs=hmT_bf[:, n:n + 1],
                                 start=(n == 0), stop=(n == NCH - 1))
            mwP = smallp.tile([128, 1], f32, tag="mwP", name=f"mwP_{bl}")
            nc.scalar.activation(out=mwP, in_=ps_mwp, func=AF.Tanh, bias=bSm)

            # ---- scoring: s = whD^T (tanh(WS hyp^T + bSp) * mwP); softmax ----
            s_exp = seqp.tile([8, T], bf16, tag="s_exp", name=f"s_exp_{bl}")
            ssum_parts = smallp.tile([8, TCH], f32, tag="ssum_parts",
                                     name=f"ssum_parts_{bl}")
            for tci in range(TCH):
                tsl = slice(tci * TC, (tci + 1) * TC)
                ps = psA.tile([128, TC], f32, tag="psA", name=f"psA_{bl}_{tci}")
                for n in range(NCH):
                    nc.tensor.matmul(ps, lhsT=WST[:, n, :],
                                     rhs=hT[n][:, tsl],
                                     start=(n == 0), stop=(n == NCH - 1))
                g1 = gp.tile([128, TC], f32, tag="g1", name=f"g1_{bl}_{tci}")
                nc.scalar.activation(out=g1, in_=ps, func=AF.Tanh, bias=bSp)
                g2 = gp.tile([128, TC], bf16, tag="g2", name=f"g2_{bl}_{tci}")
                nc.vector.tensor_scalar_mul(g2, g1, mwP)
                ps_s = psS.tile([8, TC], f32, tag="psS", name=f"ps_s_{bl}_{tci}")
                nc.tensor.matmul(ps_s, lhsT=whD, rhs=g2, start=True, stop=True)
                nc.scalar.activation(out=s_exp[:, tsl], in_=ps_s, func=AF.Exp,
                                     accum_out=ssum_parts[:, tci:tci + 1])
            ssum = smallp.tile([8, 1], f32, tag="ssum", name=f"ssum_{bl}")
            nc.vector.reduce_sum(out=ssum, in_=ssum_parts, axis=AX.X)
            sinv = smallp.tile([8, 1], f32, tag="sinv", name=f"sinv_{bl}")
            nc.vector.reciprocal(sinv, ssum)

            # ---- v = a^T hyp ; c^T = v @ Wmh^T + bmh ----
            aT = []
            for t in range(T128):
                ps_aT = psS.tile([128, 8], bf16, tag="psS",
                                 name=f"ps_aT_{bl}_{t}")
                nc.tensor.transpose(ps_aT, s_exp[:, t * 128:(t + 1) * 128],
                                    ident[:8, :8])
                aTt = aTp.tile([128, 8], bf16, tag="aT", name=f"aT_{bl}_{t}")
                nc.scalar.copy(aTt, ps_aT)
                aT.append(aTt)
            v_sb = smallp.tile([8, N], bf16, tag="v_sb", name=f"v_sb_{bl}")
            for nh in range(2):
                ps_v = psV.tile([8, 512], f32, tag="psV",
                                name=f"ps_v_{bl}_{nh}")
                for t in range(T128):
                    nc.tensor.matmul(ps_v, lhsT=aT[t],
                                     rhs=hN[t][:, nh * 512:(nh + 1) * 512],
                                     start=(t == 0), stop=(t == T128 - 1))
                nc.scalar.activation(out=v_sb[:, nh * 512:(nh + 1) * 512],
                                     in_=ps_v, func=AF.Copy, scale=sinv)
            vT = smallp.tile([128, NCH, 8], bf16, tag="vT", name=f"vT_{bl}")
            for n in range(NCH):
                ps_vT = psS.tile([128, 8], bf16, tag="psS",
                                 name=f"ps_vT_{bl}_{n}")
                nc.tensor.transpose(ps_vT, v_sb[:, n * 128:(n + 1) * 128],
                                    ident[:8, :8])
                nc.scalar.copy(vT[:, n, :], ps_vT)
            ps_cT = psS.tile([128, H], f32, tag="psS", name=f"ps_cT_{bl}")
            for h in range(H):
                for n in range(NCH):
                    nc.tensor.matmul(ps_cT[:, h:h + 1], lhsT=wmhT[:, h, n, :],
                                     rhs=vT[:, n, h:h + 1],
                                     start=(n == 0), stop=(n == NCH - 1),
                                     skip_group_check=True)
            c2 = smallp.tile([128, H], f32, tag="c2", name=f"c2_{bl}")
            nc.vector.tensor_tensor(out=c2, in0=ps_cT, in1=bmhT, op=OP.add)
            nc.sync.dma_start(out=out_d[bl], in_=c2)

    nc.compile()
    return nc


def _prep_inputs(hyp, Wmh, bmh, W, bW, Wm, bWm, Wh, bWh):
    """Host-side sharding + layout prep (numpy only)."""
    bf = ml_dtypes.bfloat16
    hyp = np.asarray(hyp, np.float32)
    Wmh = np.asarray(Wmh, np.float32)
    bmh = np.asarray(bmh, np.float32)
    W = np.asarray(W, np.float32)
    bW = np.asarray(bW, np.float32)
    Wm = np.asarray(Wm, np.float32)
    bWm = np.asarray(bWm, np.float32)
    Wh = np.asarray(Wh, np.float32)

    hyp_b = np.ascontiguousarray(hyp.transpose(1, 0, 2))          # (B, T, N)
    hypN_all = hyp_b.astype(bf)
    hypT_all = np.ascontiguousarray(hyp_b.transpose(0, 2, 1)).astype(bf)

    # Wmh (H, K, N) -> (128, H, NCH, K): [p, h, n, k] = Wmh[h, k, n*128+p]
    wmhT = np.ascontiguousarray(
        Wmh.transpose(2, 0, 1).reshape(NCH, 128, H, K)
        .transpose(1, 2, 0, 3)).astype(bf)

    # fused scoring weights: WS[h*16+q, n] = sum_k W[q,k] Wmh[h,k,n]
    WS = np.einsum('qk,hkn->hqn', W, Wmh).reshape(128, N)
    WST = np.ascontiguousarray(
        WS.T.reshape(NCH, 128, 128).transpose(1, 0, 2)).astype(bf)
    bSp = (np.einsum('qk,hk->hq', W, bmh).reshape(128)
           + np.tile(bW, H)).astype(np.float32).reshape(128, 1)

    WSm = np.einsum('qk,hkn->hqn', Wm, Wmh).reshape(128, N)
    WSmT = np.ascontiguousarray(
        WSm.T.reshape(NCH, 128, 128).transpose(1, 0, 2)).astype(bf)
    bSm = (np.einsum('qk,hk->hq', Wm, bmh).reshape(128)
           + np.tile(bWm, H)).astype(np.float32).reshape(128, 1)

    whD = np.zeros((K, H), dtype=np.float32)
    for h in range(H):
        whD[h * K2:(h + 1) * K2, h] = Wh
    whD = whD.astype(bf)
    bmhT = np.ascontiguousarray(bmh.T)                            # (K, H)

    in_maps = []
    for c in range(NCORES):
        sl = slice(c * BL, (c + 1) * BL)
        in_maps.append({
            "hypT": np.ascontiguousarray(hypT_all[sl]),
            "hypN": np.ascontiguousarray(hypN_all[sl]),
            "wmhT": wmhT, "WST": WST, "WSmT": WSmT, "whD": whD,
            "bSp": bSp, "bSm": bSm, "bmhT": bmhT,
        })
    return in_maps


def kernel(hyp, Wmh, bmh, W, bW, Wm, bWm, Wh, bWh,
           dan_hidden_size=None, attention_hidden_size=None,
           multihead_size=None, **_):
    from concourse.bass_utils import run_bass_kernel_spmd

    in_maps = _prep_inputs(hyp, Wmh, bmh, W, bW, Wm, bWm, Wh, bWh)
    if "nc" not in _cache:
        _cache["nc"] = _build_nc()
    res = run_bass_kernel_spmd(_cache["nc"], in_maps, core_ids=list(range(NCORES)))
    # out is (BL, K, H) per core -> (B, H, K) -> (B, N)
    out = np.concatenate([r["out"].transpose(0, 2, 1).reshape(BL, N)
                          for r in res.results], axis=0)
    return out.astype(np.float32)


# revision 23
# speedup vs baseline: 1.0547x; 1.0417x over previous
"""Trainium2 Bass kernel for nn_Attention_46454366273781 (sparse_attention).

Reference computation (T=2048, B=32, N=1024, H=8, K=128, K2=16):
    X = einsum('tbn,hkn->bthk', hyp, Wmh) + bmh          # per-head projections
    m = X.mean(axis=1)                                   # mean over time
    g = tanh(X @ W.T + bW) * tanh(m @ Wm.T + bWm)[:,None]
    s = g @ Wh + bWh ; a = softmax(s, axis=time)
    c = einsum('bth,bthk->bhk', a, X) ; out = c.reshape(B, H*K)

Key algebra: X itself is never needed on device.
  * scoring:  X @ W.T + bW  =  hyp @ WS.T + bSp   with WS = W @ Wmh (per head)
  * gate:     m @ Wm.T + bWm = mean_t(hyp) @ WSm.T + bSm,  WSm = Wm @ Wmh
  * output:   softmax weights sum to 1, so
              c_bh = (a_bh^T hyp_b) @ Wmh_h^T + bmh_h  (the v-trick)
This turns the 137-GFLOP projection into a rank-128 scoring matmul plus two
passes over hyp (one N-major for scoring/mean, one T-major for the weighted
sum), making the kernel DMA-bound.  Sharding: data-parallel over batch B
across 8 cores (4 batches/core).  bWh cancels inside the softmax.
"""

import numpy as np
import ml_dtypes

T, B, N, H = 2048, 32, 1024, 8
K, K2 = 128, 16          # per-head dim, attention hidden per head
NCORES = 8
BL = B // NCORES         # batches per core
TC = 512                 # time chunk for scoring matmul free dim
NCH = N // 128           # contraction chunks over N
TCH = T // TC            # time chunks (scoring)
T128 = T // 128          # 128-sized time chunks

_cache = {}


def _build_nc():
    import concourse.mybir as mybir
    import concourse.tile as tile
    from concourse import bacc
    from concourse.masks import make_identity

    bf16 = mybir.dt.bfloat16
    f32 = mybir.dt.float32
    AF = mybir.ActivationFunctionType
    AX = mybir.AxisListType
    OP = mybir.AluOpType

    nc = bacc.Bacc("TRN2")
    hypT = nc.dram_tensor("hypT", (BL, N, T), bf16, kind="ExternalInput")
    hypN = nc.dram_tensor("hypN", (BL, T, N), bf16, kind="ExternalInput")
    wmhT_d = nc.dram_tensor("wmhT", (128, H, NCH, K), bf16, kind="ExternalInput")
    WST_d = nc.dram_tensor("WST", (128, NCH, 128), bf16, kind="ExternalInput")
    WSmT_d = nc.dram_tensor("WSmT", (128, NCH, 128), bf16, kind="ExternalInput")
    whD_d = nc.dram_tensor("whD", (K, H), bf16, kind="ExternalInput")
    bSp_d = nc.dram_tensor("bSp", (128, 1), f32, kind="ExternalInput")
    bSm_d = nc.dram_tensor("bSm", (128, 1), f32, kind="ExternalInput")
    bmhT_d = nc.dram_tensor("bmhT", (K, H), f32, kind="ExternalInput")
    out_d = nc.dram_tensor("out", (BL, K, H), f32, kind="ExternalOutput")

    with tile.TileContext(nc) as tc, \
         tc.tile_pool(name="wpool", bufs=1) as wpool, \
         tc.tile_pool(name="hypTp", bufs=2 * NCH) as hypTp, \
         tc.tile_pool(name="hypNp", bufs=2 * T128) as hypNp, \
         tc.tile_pool(name="gp", bufs=3) as gp, \
         tc.tile_pool(name="aTp", bufs=2 * T128) as aTp, \
         tc.tile_pool(name="seqp", bufs=2) as seqp, \
         tc.tile_pool(name="smallp", bufs=2) as smallp, \
         tc.tile_pool(name="psA", bufs=2, space="PSUM") as psA, \
         tc.tile_pool(name="psV", bufs=2, space="PSUM") as psV, \
         tc.tile_pool(name="psS", bufs=4, space="PSUM") as psS:

        # ---- constants / weights (loaded once; wmhT last, c-phase only) ----
        WST = wpool.tile([128, NCH, 128], bf16)
        nc.sync.dma_start(out=WST, in_=WST_d[:])
        WSmT = wpool.tile([128, NCH, 128], bf16)
        nc.sync.dma_start(out=WSmT, in_=WSmT_d[:])
        whD = wpool.tile([K, H], bf16)
        nc.sync.dma_start(out=whD, in_=whD_d[:])
        bSp = wpool.tile([128, 1], f32)
        nc.sync.dma_start(out=bSp, in_=bSp_d[:])
        bSm = wpool.tile([128, 1], f32)
        nc.sync.dma_start(out=bSm, in_=bSm_d[:])
        bmhT = wpool.tile([K, H], f32)
        nc.sync.dma_start(out=bmhT, in_=bmhT_d[:])
        ident = wpool.tile([128, 128], bf16)
        make_identity(nc, ident)
        wmhT = wpool.tile([128, H, NCH, K], bf16)
        nc.sync.dma_start(out=wmhT, in_=wmhT_d[:])
        dump = wpool.tile([128, T], bf16)   # write-only sink for mean pass

        for bl in range(BL):
            # ---- load hyp in both layouts ----
            hT = [hypTp.tile([128, T], bf16, tag="hT", name=f"hT_{bl}_{i}")
                  for i in range(NCH)]
            for n in range(NCH):
                nc.sync.dma_start(out=hT[n],
                                  in_=hypT[bl, n * 128:(n + 1) * 128, :])
            hN = [hypNp.tile([128, N], bf16, tag="hN", name=f"hN_{bl}_{i}")
                  for i in range(T128)]
            for t in range(T128):
                nc.sync.dma_start(out=hN[t],
                                  in_=hypN[bl, t * 128:(t + 1) * 128, :])

            # ---- gate: mw = tanh(WSm @ mean_t(hyp)^T + bSm), packed [hq,1] ----
            hmT = smallp.tile([128, NCH], f32, tag="hmT", name=f"hmT_{bl}")
            for n in range(NCH):
                if n % 2 == 0:
                    nc.scalar.activation(out=dump, in_=hT[n], func=AF.Copy,
                                         accum_out=hmT[:, n:n + 1])
                else:
                    nc.vector.reduce_sum(out=hmT[:, n:n + 1], in_=hT[n],
                                         axis=AX.X)
            hmT_bf = smallp.tile([128, NCH], bf16, tag="hmT_bf",
                                 name=f"hmT_bf_{bl}")
            nc.scalar.activation(out=hmT_bf, in_=hmT, func=AF.Copy,
                                 scale=1.0 / T)
            ps_mwp = psS.tile([128, 1], f32, tag="psS", name=f"ps_mwp_{bl}")
            for n in range(NCH):
                nc.tensor.matmul(ps_mwp, lhsT=WSmT[:, n, :],
                                 rhs=hmT_bf[:, n:n + 1],
                                 start=(n == 0), stop=(n == NCH - 1))
            mwP = smallp.tile([128, 1], f32, tag="mwP", name=f"mwP_{bl}")
            nc.scalar.activation(out=mwP, in_=ps_mwp, func=AF.Tanh, bias=bSm)

            # ---- scoring: s = whD^T (tanh(WS hyp^T + bSp) * mwP); softmax ----
            s_exp = seqp.tile([8, T], bf16, tag="s_exp", name=f"s_exp_{bl}")
            ssum_parts = smallp.tile([8, TCH], f32, tag="ssum_parts",
                                     name=f"ssum_parts_{bl}")
            for tci in range(TCH):
                tsl = slice(tci * TC, (tci + 1) * TC)
                ps = psA.tile([128, TC], f32, tag="psA", name=f"psA_{bl}_{tci}")
                for n in range(NCH):
                    nc.tensor.matmul(ps, lhsT=WST[:, n, :],
                                     rhs=hT[n][:, tsl],
                                     start=(n == 0), stop=(n == NCH - 1))
                g1 = gp.tile([128, TC], f32, tag="g1", name=f"g1_{bl}_{tci}")
                nc.scalar.activation(out=g1, in_=ps, func=AF.Tanh, bias=bSp)
                g2 = gp.tile([128, TC], bf16, tag="g2", name=f"g2_{bl}_{tci}")
                nc.vector.tensor_scalar_mul(g2, g1, mwP)
                ps_s = psS.tile([8, TC], f32, tag="psS", name=f"ps_s_{bl}_{tci}")
                nc.tensor.matmul(ps_s, lhsT=whD, rhs=g2, start=True, stop=True)
                nc.scalar.activation(out=s_exp[:, tsl], in_=ps_s, func=AF.Exp,
                                     accum_out=ssum_parts[:, tci:tci + 1])
            ssum = smallp.tile([8, 1], f32, tag="ssum", name=f"ssum_{bl}")
            nc.vector.reduce_sum(out=ssum, in_=ssum_parts, axis=AX.X)
            sinv = smallp.tile([8, 1], f32, tag="sinv", name=f"sinv_{bl}")
            nc.vector.reciprocal(sinv, ssum)

            # ---- v = a^T hyp ; c^T = v @ Wmh^T + bmh ----
            aT = []
            for t in range(T128):
                ps_aT = psS.tile([128, 8], bf16, tag="psS",
                                 name=f"ps_aT_{bl}_{t}")
                nc.tensor.transpose(ps_aT, s_exp[:, t * 128:(t + 1) * 128],
                                    ident[:8, :8])
                aTt = aTp.tile([128, 8], bf16, tag="aT", name=f"aT_{bl}_{t}")
                nc.scalar.copy(aTt, ps_aT)
                aT.append(aTt)
            v_sb = smallp.tile([8, N], bf16, tag="v_sb", name=f"v_sb_{bl}")
            for nh in range(2):
                ps_v = psV.tile([8, 512], f32, tag="psV",
                                name=f"ps_v_{bl}_{nh}")
                for t in range(T128):
                    nc.tensor.matmul(ps_v, lhsT=aT[t],
                                     rhs=hN[t][:, nh * 512:(nh + 1) * 512],
                                     start=(t == 0), stop=(t == T128 - 1))
                nc.scalar.activation(out=v_sb[:, nh * 512:(nh + 1) * 512],
                                     in_=ps_v, func=AF.Copy, scale=sinv)
            vT = smallp.tile([128, NCH, 8], bf16, tag="vT", name=f"vT_{bl}")
            for n in range(NCH):
                ps_vT = psS.tile([128, 8], bf16, tag="psS",
                                 name=f"ps_vT_{bl}_{n}")
                nc.tensor.transpose(ps_vT, v_sb[:, n * 128:(n + 1) * 128],
                                    ident[:8, :8])
                nc.scalar.copy(vT[:, n, :], ps_vT)
            ps_cT = psS.tile([128, H], f32, tag="psS", name=f"ps_cT_{bl}")
            for h in range(H):
                for n in range(NCH):
                    nc.tensor.matmul(ps_cT[:, h:h + 1], lhsT=wmhT[:, h, n, :],
                                     rhs=vT[:, n, h:h + 1],
                                     start=(n == 0), stop=(n == NCH - 1),
                                     skip_group_check=True)
            c2 = smallp.tile([128, H], f32, tag="c2", name=f"c2_{bl}")
            nc.vector.tensor_tensor(out=c2, in0=ps_cT, in1=bmhT, op=OP.add)
            nc.sync.dma_start(out=out_d[bl], in_=c2)

    nc.compile()
    return nc


def _prep_inputs(hyp, Wmh, bmh, W, bW, Wm, bWm, Wh, bWh):
    """Host-side sharding + layout prep (numpy only)."""
    bf = ml_dtypes.bfloat16
    hyp = np.asarray(hyp, np.float32)
    Wmh = np.asarray(Wmh, np.float32)
    bmh = np.asarray(bmh, np.float32)
    W = np.asarray(W, np.float32)
    bW = np.asarray(bW, np.float32)
    Wm = np.asarray(Wm, np.float32)
    bWm = np.asarray(bWm, np.float32)
    Wh = np.asarray(Wh, np.float32)

    hyp_b = np.ascontiguousarray(hyp.transpose(1, 0, 2))          # (B, T, N)
    hypN_all = hyp_b.astype(bf)
    hypT_all = np.ascontiguousarray(hyp_b.transpose(0, 2, 1)).astype(bf)

    # Wmh (H, K, N) -> (128, H, NCH, K): [p, h, n, k] = Wmh[h, k, n*128+p]
    wmhT = np.ascontiguousarray(
        Wmh.transpose(2, 0, 1).reshape(NCH, 128, H, K)
        .transpose(1, 2, 0, 3)).astype(bf)

    # fused scoring weights: WS[h*16+q, n] = sum_k W[q,k] Wmh[h,k,n]
    WS = np.einsum('qk,hkn->hqn', W, Wmh).reshape(128, N)
    WST = np.ascontiguousarray(
        WS.T.reshape(NCH, 128, 128).transpose(1, 0, 2)).astype(bf)
    bSp = (np.einsum('qk,hk->hq', W, bmh).reshape(128)
           + np.tile(bW, H)).astype(np.float32).reshape(128, 1)

    WSm = np.einsum('qk,hkn->hqn', Wm, Wmh).reshape(128, N)
    WSmT = np.ascontiguousarray(
        WSm.T.reshape(NCH, 128, 128).transpose(1, 0, 2)).astype(bf)
    bSm = (np.einsum('qk,hk->hq', Wm, bmh).reshape(128)
           + np.tile(bWm, H)).astype(np.float32).reshape(128, 1)

    whD = np.zeros((K, H), dtype=np.float32)
    for h in range(H):
        whD[h * K2:(h + 1) * K2, h] = Wh
    whD = whD.astype(bf)
    bmhT = np.ascontiguousarray(bmh.T)                            # (K, H)

    in_maps = []
    for c in range(NCORES):
        sl = slice(c * BL, (c + 1) * BL)
        in_maps.append({
            "hypT": np.ascontiguousarray(hypT_all[sl]),
            "hypN": np.ascontiguousarray(hypN_all[sl]),
            "wmhT": wmhT, "WST": WST, "WSmT": WSmT, "whD": whD,
            "bSp": bSp, "bSm": bSm, "bmhT": bmhT,
        })
    return in_maps


def kernel(hyp, Wmh, bmh, W, bW, Wm, bWm, Wh, bWh,
           dan_hidden_size=None, attention_hidden_size=None,
           multihead_size=None, **_):
    from concourse.bass_utils import run_bass_kernel_spmd

    in_maps = _prep_inputs(hyp, Wmh, bmh, W, bW, Wm, bWm, Wh, bWh)
    if "nc" not in _cache:
        _cache["nc"] = _build_nc()
    res = run_bass_kernel_spmd(_cache["nc"], in_maps, core_ids=list(range(NCORES)))
    # out is (BL, K, H) per core -> (B, H, K) -> (B, N)
    out = np.concatenate([r["out"].transpose(0, 2, 1).reshape(BL, N)
                          for r in res.results], axis=0)
    return out.astype(np.float32)


# revision 24
# speedup vs baseline: 1.0837x; 1.0275x over previous
"""Trainium2 Bass kernel for nn_Attention_46454366273781 (sparse_attention).

Reference computation (T=2048, B=32, N=1024, H=8, K=128, K2=16):
    X = einsum('tbn,hkn->bthk', hyp, Wmh) + bmh          # per-head projections
    m = X.mean(axis=1)                                   # mean over time
    g = tanh(X @ W.T + bW) * tanh(m @ Wm.T + bWm)[:,None]
    s = g @ Wh + bWh ; a = softmax(s, axis=time)
    c = einsum('bth,bthk->bhk', a, X) ; out = c.reshape(B, H*K)

Key algebra: X itself is never needed on device.
  * scoring:  X @ W.T + bW  =  hyp @ WS.T + bSp   with WS = W @ Wmh (per head)
  * gate:     m @ Wm.T + bWm = mean_t(hyp) @ WSm.T + bSm,  WSm = Wm @ Wmh
  * output:   softmax weights sum to 1, so
              c_bh = (a_bh^T hyp_b) @ Wmh_h^T + bmh_h  (the v-trick)
This turns the 137-GFLOP projection into a rank-128 scoring matmul plus two
passes over hyp (one N-major for scoring/mean, one T-major for the weighted
sum), making the kernel DMA-bound.  Sharding: data-parallel over batch B
across 8 cores (4 batches/core).  bWh cancels inside the softmax.
"""

import numpy as np
import ml_dtypes

T, B, N, H = 2048, 32, 1024, 8
K, K2 = 128, 16          # per-head dim, attention hidden per head
NCORES = 8
BL = B // NCORES         # batches per core
TC = 512                 # time chunk for scoring matmul free dim
NCH = N // 128           # contraction chunks over N
TCH = T // TC            # time chunks (scoring)
T128 = T // 128          # 128-sized time chunks

_cache = {}


def _build_nc():
    import concourse.mybir as mybir
    import concourse.tile as tile
    from concourse import bacc
    from concourse.masks import make_identity

    bf16 = mybir.dt.bfloat16
    f32 = mybir.dt.float32
    AF = mybir.ActivationFunctionType
    AX = mybir.AxisListType
    OP = mybir.AluOpType

    nc = bacc.Bacc("TRN2")
    hypT = nc.dram_tensor("hypT", (BL, N, T), bf16, kind="ExternalInput")
    hypN = nc.dram_tensor("hypN", (BL, T, N), bf16, kind="ExternalInput")
    wmhT_d = nc.dram_tensor("wmhT", (128, H, NCH, K), bf16, kind="ExternalInput")
    WST_d = nc.dram_tensor("WST", (128, NCH, 128), bf16, kind="ExternalInput")
    WSmT_d = nc.dram_tensor("WSmT", (128, NCH, 128), bf16, kind="ExternalInput")
    whD_d = nc.dram_tensor("whD", (K, H), bf16, kind="ExternalInput")
    bSp_d = nc.dram_tensor("bSp", (128, 1), f32, kind="ExternalInput")
    bSm_d = nc.dram_tensor("bSm", (128, 1), f32, kind="ExternalInput")
    bmhT_d = nc.dram_tensor("bmhT", (K, H), f32, kind="ExternalInput")
    out_d = nc.dram_tensor("out", (BL, K, H), f32, kind="ExternalOutput")

    with tile.TileContext(nc) as tc, \
         tc.tile_pool(name="wpool", bufs=1) as wpool, \
         tc.tile_pool(name="hypTp", bufs=2 * NCH) as hypTp, \
         tc.tile_pool(name="hypNp", bufs=2 * T128) as hypNp, \
         tc.tile_pool(name="gp", bufs=3) as gp, \
         tc.tile_pool(name="aTp", bufs=2 * T128) as aTp, \
         tc.tile_pool(name="seqp", bufs=2) as seqp, \
         tc.tile_pool(name="smallp", bufs=2) as smallp, \
         tc.tile_pool(name="psA", bufs=2, space="PSUM") as psA, \
         tc.tile_pool(name="psV", bufs=2, space="PSUM") as psV, \
         tc.tile_pool(name="psS", bufs=4, space="PSUM") as psS:

        # ---- constants / weights (loaded once; wmhT last, c-phase only) ----
        WST = wpool.tile([128, NCH, 128], bf16)
        nc.gpsimd.dma_start(out=WST, in_=WST_d[:])
        WSmT = wpool.tile([128, NCH, 128], bf16)
        nc.gpsimd.dma_start(out=WSmT, in_=WSmT_d[:])
        whD = wpool.tile([K, H], bf16)
        nc.gpsimd.dma_start(out=whD, in_=whD_d[:])
        bSp = wpool.tile([128, 1], f32)
        nc.gpsimd.dma_start(out=bSp, in_=bSp_d[:])
        bSm = wpool.tile([128, 1], f32)
        nc.gpsimd.dma_start(out=bSm, in_=bSm_d[:])
        bmhT = wpool.tile([K, H], f32)
        nc.gpsimd.dma_start(out=bmhT, in_=bmhT_d[:])
        ident = wpool.tile([128, 128], bf16)
        make_identity(nc, ident)
        wmhT = wpool.tile([128, H, NCH, K], bf16)
        nc.gpsimd.dma_start(out=wmhT, in_=wmhT_d[:])
        dump = wpool.tile([128, T], bf16)   # write-only sink for mean pass

        for bl in range(BL):
            # ---- load hyp in both layouts ----
            hT = [hypTp.tile([128, T], bf16, tag="hT", name=f"hT_{bl}_{i}")
                  for i in range(NCH)]
            for n in range(NCH):
                nc.sync.dma_start(out=hT[n],
                                  in_=hypT[bl, n * 128:(n + 1) * 128, :])
            hN = [hypNp.tile([128, N], bf16, tag="hN", name=f"hN_{bl}_{i}")
                  for i in range(T128)]
            for t in range(T128):
                nc.sync.dma_start(out=hN[t],
                                  in_=hypN[bl, t * 128:(t + 1) * 128, :])

            # ---- gate: mw = tanh(WSm @ mean_t(hyp)^T + bSm), packed [hq,1] ----
            hmT = smallp.tile([128, NCH], f32, tag="hmT", name=f"hmT_{bl}")
            for n in range(NCH):
                if n % 2 == 0:
                    nc.scalar.activation(out=dump, in_=hT[n], func=AF.Copy,
                                         accum_out=hmT[:, n:n + 1])
                else:
                    nc.vector.reduce_sum(out=hmT[:, n:n + 1], in_=hT[n],
                                         axis=AX.X)
            hmT_bf = smallp.tile([128, NCH], bf16, tag="hmT_bf",
                                 name=f"hmT_bf_{bl}")
            nc.scalar.activation(out=hmT_bf, in_=hmT, func=AF.Copy,
                                 scale=1.0 / T)
            ps_mwp = psS.tile([128, 1], f32, tag="psS", name=f"ps_mwp_{bl}")
            for n in range(NCH):
                nc.tensor.matmul(ps_mwp, lhsT=WSmT[:, n, :],
                                 rhs=hmT_bf[:, n:n + 1],
                                 start=(n == 0), stop=(n == NCH - 1))
            mwP = smallp.tile([128, 1], f32, tag="mwP", name=f"mwP_{bl}")
            nc.scalar.activation(out=mwP, in_=ps_mwp, func=AF.Tanh, bias=bSm)

            # ---- scoring: s = whD^T (tanh(WS hyp^T + bSp) * mwP); softmax ----
            s_exp = seqp.tile([8, T], bf16, tag="s_exp", name=f"s_exp_{bl}")
            ssum_parts = smallp.tile([8, TCH], f32, tag="ssum_parts",
                                     name=f"ssum_parts_{bl}")
            for tci in range(TCH):
                tsl = slice(tci * TC, (tci + 1) * TC)
                ps = psA.tile([128, TC], f32, tag="psA", name=f"psA_{bl}_{tci}")
                for n in range(NCH):
                    nc.tensor.matmul(ps, lhsT=WST[:, n, :],
                                     rhs=hT[n][:, tsl],
                                     start=(n == 0), stop=(n == NCH - 1))
                g1 = gp.tile([128, TC], f32, tag="g1", name=f"g1_{bl}_{tci}")
                nc.scalar.activation(out=g1, in_=ps, func=AF.Tanh, bias=bSp)
                g2 = gp.tile([128, TC], bf16, tag="g2", name=f"g2_{bl}_{tci}")
                nc.vector.tensor_scalar_mul(g2, g1, mwP)
                ps_s = psS.tile([8, TC], f32, tag="psS", name=f"ps_s_{bl}_{tci}")
                nc.tensor.matmul(ps_s, lhsT=whD, rhs=g2, start=True, stop=True)
                nc.scalar.activation(out=s_exp[:, tsl], in_=ps_s, func=AF.Exp,
                                     accum_out=ssum_parts[:, tci:tci + 1])
            ssum = smallp.tile([8, 1], f32, tag="ssum", name=f"ssum_{bl}")
            nc.vector.reduce_sum(out=ssum, in_=ssum_parts, axis=AX.X)
            sinv = smallp.tile([8, 1], f32, tag="sinv", name=f"sinv_{bl}")
            nc.vector.reciprocal(sinv, ssum)

            # ---- v = a^T hyp ; c^T = v @ Wmh^T + bmh ----
            aT = []
            for t in range(T128):
                ps_aT = psS.tile([128, 8], bf16, tag="psS",
                                 name=f"ps_aT_{bl}_{t}")
                nc.tensor.transpose(ps_aT, s_exp[:, t * 128:(t + 1) * 128],
                                    ident[:8, :8])
                aTt = aTp.tile([128, 8], bf16, tag="aT", name=f"aT_{bl}_{t}")
                nc.scalar.copy(aTt, ps_aT)
                aT.append(aTt)
            v_sb = smallp.tile([8, N], bf16, tag="v_sb", name=f"v_sb_{bl}")
            for nh in range(2):
                ps_v = psV.tile([8, 512], f32, tag="psV",
                                name=f"ps_v_{bl}_{nh}")
                for t in range(T128):
                    nc.tensor.matmul(ps_v, lhsT=aT[t],
                                     rhs=hN[t][:, nh * 512:(nh + 1) * 512],
                                     start=(t == 0), stop=(t == T128 - 1))
                nc.scalar.activation(out=v_sb[:, nh * 512:(nh + 1) * 512],
                                     in_=ps_v, func=AF.Copy, scale=sinv)
            vT = smallp.tile([128, NCH, 8], bf16, tag="vT", name=f"vT_{bl}")
            for n in range(NCH):
                ps_vT = psS.tile([128, 8], bf16, tag="psS",
                                 name=f"ps_vT_{bl}_{n}")
                nc.tensor.transpose(ps_vT, v_sb[:, n * 128:(n + 1) * 128],
                                    ident[:8, :8])
                nc.scalar.copy(vT[:, n, :], ps_vT)
            ps_cT = psS.tile([128, H], f32, tag="psS", name=f"ps_cT_{bl}")
            for h in range(H):
                for n in range(NCH):
                    nc.tensor.matmul(ps_cT[:, h:h + 1], lhsT=wmhT[:, h, n, :],
                                     rhs=vT[:, n, h:h + 1],
                                     start=(n == 0), stop=(n == NCH - 1),
                                     skip_group_check=True)
            c2 = smallp.tile([128, H], f32, tag="c2", name=f"c2_{bl}")
            nc.vector.tensor_tensor(out=c2, in0=ps_cT, in1=bmhT, op=OP.add)
            nc.sync.dma_start(out=out_d[bl], in_=c2)

    nc.compile()
    return nc


def _prep_inputs(hyp, Wmh, bmh, W, bW, Wm, bWm, Wh, bWh):
    """Host-side sharding + layout prep (numpy only)."""
    bf = ml_dtypes.bfloat16
    hyp = np.asarray(hyp, np.float32)
    Wmh = np.asarray(Wmh, np.float32)
    bmh = np.asarray(bmh, np.float32)
    W = np.asarray(W, np.float32)
    bW = np.asarray(bW, np.float32)
    Wm = np.asarray(Wm, np.float32)
    bWm = np.asarray(bWm, np.float32)
    Wh = np.asarray(Wh, np.float32)

    hyp_b = np.ascontiguousarray(hyp.transpose(1, 0, 2))          # (B, T, N)
    hypN_all = hyp_b.astype(bf)
    hypT_all = np.ascontiguousarray(hyp_b.transpose(0, 2, 1)).astype(bf)

    # Wmh (H, K, N) -> (128, H, NCH, K): [p, h, n, k] = Wmh[h, k, n*128+p]
    wmhT = np.ascontiguousarray(
        Wmh.transpose(2, 0, 1).reshape(NCH, 128, H, K)
        .transpose(1, 2, 0, 3)).astype(bf)

    # fused scoring weights: WS[h*16+q, n] = sum_k W[q,k] Wmh[h,k,n]
    WS = np.einsum('qk,hkn->hqn', W, Wmh).reshape(128, N)
    WST = np.ascontiguousarray(
        WS.T.reshape(NCH, 128, 128).transpose(1, 0, 2)).astype(bf)
    bSp = (np.einsum('qk,hk->hq', W, bmh).reshape(128)
           + np.tile(bW, H)).astype(np.float32).reshape(128, 1)

    WSm = np.einsum('qk,hkn->hqn', Wm, Wmh).reshape(128, N)
    WSmT = np.ascontiguousarray(
        WSm.T.reshape(NCH, 128, 128).transpose(1, 0, 2)).astype(bf)
    bSm = (np.einsum('qk,hk->hq', Wm, bmh).reshape(128)
           + np.tile(bWm, H)).astype(np.float32).reshape(128, 1)

    whD = np.zeros((K, H), dtype=np.float32)
    for h in range(H):
        whD[h * K2:(h + 1) * K2, h] = Wh
    whD = whD.astype(bf)
    bmhT = np.ascontiguousarray(bmh.T)                            # (K, H)

    in_maps = []
    for c in range(NCORES):
        sl = slice(c * BL, (c + 1) * BL)
        in_maps.append({
            "hypT": np.ascontiguousarray(hypT_all[sl]),
            "hypN": np.ascontiguousarray(hypN_all[sl]),
            "wmhT": wmhT, "WST": WST, "WSmT": WSmT, "whD": whD,
            "bSp": bSp, "bSm": bSm, "bmhT": bmhT,
        })
    return in_maps


def kernel(hyp, Wmh, bmh, W, bW, Wm, bWm, Wh, bWh,
           dan_hidden_size=None, attention_hidden_size=None,
           multihead_size=None, **_):
    from concourse.bass_utils import run_bass_kernel_spmd

    in_maps = _prep_inputs(hyp, Wmh, bmh, W, bW, Wm, bWm, Wh, bWh)
    if "nc" not in _cache:
        _cache["nc"] = _build_nc()
    res = run_bass_kernel_spmd(_cache["nc"], in_maps, core_ids=list(range(NCORES)))
    # out is (BL, K, H) per core -> (B, H, K) -> (B, N)
    out = np.concatenate([r["out"].transpose(0, 2, 1).reshape(BL, N)
                          for r in res.results], axis=0)
    return out.astype(np.float32)


# revision 25
# speedup vs baseline: 1.0953x; 1.0107x over previous
"""Trainium2 Bass kernel for nn_Attention_46454366273781 (sparse_attention).

Reference computation (T=2048, B=32, N=1024, H=8, K=128, K2=16):
    X = einsum('tbn,hkn->bthk', hyp, Wmh) + bmh          # per-head projections
    m = X.mean(axis=1)                                   # mean over time
    g = tanh(X @ W.T + bW) * tanh(m @ Wm.T + bWm)[:,None]
    s = g @ Wh + bWh ; a = softmax(s, axis=time)
    c = einsum('bth,bthk->bhk', a, X) ; out = c.reshape(B, H*K)

Key algebra: X itself is never needed on device.
  * scoring:  X @ W.T + bW  =  hyp @ WS.T + bSp   with WS = W @ Wmh (per head)
  * gate:     m @ Wm.T + bWm = mean_t(hyp) @ WSm.T + bSm,  WSm = Wm @ Wmh
  * output:   softmax weights sum to 1, so
              c_bh = (a_bh^T hyp_b) @ Wmh_h^T + bmh_h  (the v-trick)
This turns the 137-GFLOP projection into a rank-128 scoring matmul plus two
passes over hyp (one N-major for scoring/mean, one T-major for the weighted
sum), making the kernel DMA-bound.  Sharding: data-parallel over batch B
across 8 cores (4 batches/core).  bWh cancels inside the softmax.
"""

import numpy as np
import ml_dtypes

T, B, N, H = 2048, 32, 1024, 8
K, K2 = 128, 16          # per-head dim, attention hidden per head
NCORES = 8
BL = B // NCORES         # batches per core
TC = 512                 # time chunk for scoring matmul free dim
NCH = N // 128           # contraction chunks over N
TCH = T // TC            # time chunks (scoring)
T128 = T // 128          # 128-sized time chunks

_cache = {}


def _build_nc():
    import concourse.mybir as mybir
    import concourse.tile as tile
    from concourse import bacc
    from concourse.masks import make_identity

    bf16 = mybir.dt.bfloat16
    f32 = mybir.dt.float32
    AF = mybir.ActivationFunctionType
    AX = mybir.AxisListType
    OP = mybir.AluOpType

    nc = bacc.Bacc("TRN2")
    hypT = nc.dram_tensor("hypT", (BL, N, T), bf16, kind="ExternalInput")
    hypN = nc.dram_tensor("hypN", (BL, T, N), bf16, kind="ExternalInput")
    wmhT_d = nc.dram_tensor("wmhT", (128, H, NCH, K), bf16, kind="ExternalInput")
    WST_d = nc.dram_tensor("WST", (128, NCH, 128), bf16, kind="ExternalInput")
    WSmT_d = nc.dram_tensor("WSmT", (128, NCH, 128), bf16, kind="ExternalInput")
    whD_d = nc.dram_tensor("whD", (K, H), bf16, kind="ExternalInput")
    bSp_d = nc.dram_tensor("bSp", (128, 1), f32, kind="ExternalInput")
    bSm_d = nc.dram_tensor("bSm", (128, 1), f32, kind="ExternalInput")
    bmhT_d = nc.dram_tensor("bmhT", (K, H), f32, kind="ExternalInput")
    out_d = nc.dram_tensor("out", (BL, K, H), f32, kind="ExternalOutput")

    with tile.TileContext(nc) as tc, \
         tc.tile_pool(name="wpool", bufs=1) as wpool, \
         tc.tile_pool(name="hypTp", bufs=2 * (NCH // 2)) as hypTp, \
         tc.tile_pool(name="hypNp", bufs=2 * T128) as hypNp, \
         tc.tile_pool(name="gp", bufs=3) as gp, \
         tc.tile_pool(name="aTp", bufs=2 * T128) as aTp, \
         tc.tile_pool(name="seqp", bufs=2) as seqp, \
         tc.tile_pool(name="smallp", bufs=2) as smallp, \
         tc.tile_pool(name="psA", bufs=2, space="PSUM") as psA, \
         tc.tile_pool(name="psV", bufs=2, space="PSUM") as psV, \
         tc.tile_pool(name="psS", bufs=4, space="PSUM") as psS:

        # ---- constants / weights (loaded once; wmhT last, c-phase only) ----
        WST = wpool.tile([128, NCH, 128], bf16)
        nc.gpsimd.dma_start(out=WST, in_=WST_d[:])
        WSmT = wpool.tile([128, NCH, 128], bf16)
        nc.gpsimd.dma_start(out=WSmT, in_=WSmT_d[:])
        whD = wpool.tile([K, H], bf16)
        nc.gpsimd.dma_start(out=whD, in_=whD_d[:])
        bSp = wpool.tile([128, 1], f32)
        nc.gpsimd.dma_start(out=bSp, in_=bSp_d[:])
        bSm = wpool.tile([128, 1], f32)
        nc.gpsimd.dma_start(out=bSm, in_=bSm_d[:])
        bmhT = wpool.tile([K, H], f32)
        nc.gpsimd.dma_start(out=bmhT, in_=bmhT_d[:])
        ident = wpool.tile([128, 128], bf16)
        make_identity(nc, ident)
        wmhT = wpool.tile([128, H, NCH, K], bf16)
        nc.gpsimd.dma_start(out=wmhT, in_=wmhT_d[:])
        dump = wpool.tile([128, T], bf16)   # write-only sink for mean pass

        for bl in range(BL):
            # ---- load hyp in both layouts ----
            hTg = [hypTp.tile([128, 2, T], bf16, tag="hT",
                           name=f"hT_{bl}_{i}") for i in range(NCH // 2)]
            for i in range(NCH // 2):
                nc.sync.dma_start(
                    out=hTg[i],
                    in_=hypT[bl, i * 2 * 128:(i + 1) * 2 * 128, :]
                    .rearrange("(j p) t -> p j t", p=128))
            hT = [hTg[n // 2][:, n % 2, :] for n in range(NCH)]
            hN = [hypNp.tile([128, N], bf16, tag="hN", name=f"hN_{bl}_{i}")
                  for i in range(T128)]
            for t in range(T128):
                nc.sync.dma_start(out=hN[t],
                                  in_=hypN[bl, t * 128:(t + 1) * 128, :])

            # ---- gate: mw = tanh(WSm @ mean_t(hyp)^T + bSm), packed [hq,1] ----
            hmT = smallp.tile([128, NCH], f32, tag="hmT", name=f"hmT_{bl}")
            for n in range(NCH):
                if n % 2 == 0:
                    nc.scalar.activation(out=dump, in_=hT[n], func=AF.Copy,
                                         accum_out=hmT[:, n:n + 1])
                else:
                    nc.vector.reduce_sum(out=hmT[:, n:n + 1], in_=hT[n],
                                         axis=AX.X)
            hmT_bf = smallp.tile([128, NCH], bf16, tag="hmT_bf",
                                 name=f"hmT_bf_{bl}")
            nc.scalar.activation(out=hmT_bf, in_=hmT, func=AF.Copy,
                                 scale=1.0 / T)
            ps_mwp = psS.tile([128, 1], f32, tag="psS", name=f"ps_mwp_{bl}")
            for n in range(NCH):
                nc.tensor.matmul(ps_mwp, lhsT=WSmT[:, n, :],
                                 rhs=hmT_bf[:, n:n + 1],
                                 start=(n == 0), stop=(n == NCH - 1))
            mwP = smallp.tile([128, 1], f32, tag="mwP", name=f"mwP_{bl}")
            nc.scalar.activation(out=mwP, in_=ps_mwp, func=AF.Tanh, bias=bSm)

            # ---- scoring: s = whD^T (tanh(WS hyp^T + bSp) * mwP); softmax ----
            s_exp = seqp.tile([8, T], bf16, tag="s_exp", name=f"s_exp_{bl}")
            ssum_parts = smallp.tile([8, TCH], f32, tag="ssum_parts",
                                     name=f"ssum_parts_{bl}")
            for tci in range(TCH):
                tsl = slice(tci * TC, (tci + 1) * TC)
                ps = psA.tile([128, TC], f32, tag="psA", name=f"psA_{bl}_{tci}")
                for n in range(NCH):
                    nc.tensor.matmul(ps, lhsT=WST[:, n, :],
                                     rhs=hT[n][:, tsl],
                                     start=(n == 0), stop=(n == NCH - 1))
                g1 = gp.tile([128, TC], f32, tag="g1", name=f"g1_{bl}_{tci}")
                nc.scalar.activation(out=g1, in_=ps, func=AF.Tanh, bias=bSp)
                g2 = gp.tile([128, TC], bf16, tag="g2", name=f"g2_{bl}_{tci}")
                nc.vector.tensor_scalar_mul(g2, g1, mwP)
                ps_s = psS.tile([8, TC], f32, tag="psS", name=f"ps_s_{bl}_{tci}")
                nc.tensor.matmul(ps_s, lhsT=whD, rhs=g2, start=True, stop=True)
                nc.scalar.activation(out=s_exp[:, tsl], in_=ps_s, func=AF.Exp,
                                     accum_out=ssum_parts[:, tci:tci + 1])
            ssum = smallp.tile([8, 1], f32, tag="ssum", name=f"ssum_{bl}")
            nc.vector.reduce_sum(out=ssum, in_=ssum_parts, axis=AX.X)
            sinv = smallp.tile([8, 1], f32, tag="sinv", name=f"sinv_{bl}")
            nc.vector.reciprocal(sinv, ssum)

            # ---- v = a^T hyp ; c^T = v @ Wmh^T + bmh ----
            aT = []
            for t in range(T128):
                ps_aT = psS.tile([128, 8], bf16, tag="psS",
                                 name=f"ps_aT_{bl}_{t}")
                nc.tensor.transpose(ps_aT, s_exp[:, t * 128:(t + 1) * 128],
                                    ident[:8, :8])
                aTt = aTp.tile([128, 8], bf16, tag="aT", name=f"aT_{bl}_{t}")
                nc.scalar.copy(aTt, ps_aT)
                aT.append(aTt)
            v_sb = smallp.tile([8, N], bf16, tag="v_sb", name=f"v_sb_{bl}")
            for nh in range(2):
                ps_v = psV.tile([8, 512], f32, tag="psV",
                                name=f"ps_v_{bl}_{nh}")
                for t in range(T128):
                    nc.tensor.matmul(ps_v, lhsT=aT[t],
                                     rhs=hN[t][:, nh * 512:(nh + 1) * 512],
                                     start=(t == 0), stop=(t == T128 - 1))
                nc.scalar.activation(out=v_sb[:, nh * 512:(nh + 1) * 512],
                                     in_=ps_v, func=AF.Copy, scale=sinv)
            vT = smallp.tile([128, NCH, 8], bf16, tag="vT", name=f"vT_{bl}")
            for n in range(NCH):
                ps_vT = psS.tile([128, 8], bf16, tag="psS",
                                 name=f"ps_vT_{bl}_{n}")
                nc.tensor.transpose(ps_vT, v_sb[:, n * 128:(n + 1) * 128],
                                    ident[:8, :8])
                nc.scalar.copy(vT[:, n, :], ps_vT)
            ps_cT = psS.tile([128, H], f32, tag="psS", name=f"ps_cT_{bl}")
            for h in range(H):
                for n in range(NCH):
                    nc.tensor.matmul(ps_cT[:, h:h + 1], lhsT=wmhT[:, h, n, :],
                                     rhs=vT[:, n, h:h + 1],
                                     start=(n == 0), stop=(n == NCH - 1),
                                     skip_group_check=True)
            c2 = smallp.tile([128, H], f32, tag="c2", name=f"c2_{bl}")
            nc.vector.tensor_tensor(out=c2, in0=ps_cT, in1=bmhT, op=OP.add)
            nc.sync.dma_start(out=out_d[bl], in_=c2)

    nc.compile()
    return nc


def _prep_inputs(hyp, Wmh, bmh, W, bW, Wm, bWm, Wh, bWh):
    """Host-side sharding + layout prep (numpy only)."""
    bf = ml_dtypes.bfloat16
    hyp = np.asarray(hyp, np.float32)
    Wmh = np.asarray(Wmh, np.float32)
    bmh = np.asarray(bmh, np.float32)
    W = np.asarray(W, np.float32)
    bW = np.asarray(bW, np.float32)
    Wm = np.asarray(Wm, np.float32)
    bWm = np.asarray(bWm, np.float32)
    Wh = np.asarray(Wh, np.float32)

    hyp_b = np.ascontiguousarray(hyp.transpose(1, 0, 2))          # (B, T, N)
    hypN_all = hyp_b.astype(bf)
    hypT_all = np.ascontiguousarray(hyp_b.transpose(0, 2, 1)).astype(bf)

    # Wmh (H, K, N) -> (128, H, NCH, K): [p, h, n, k] = Wmh[h, k, n*128+p]
    wmhT = np.ascontiguousarray(
        Wmh.transpose(2, 0, 1).reshape(NCH, 128, H, K)
        .transpose(1, 2, 0, 3)).astype(bf)

    # fused scoring weights: WS[h*16+q, n] = sum_k W[q,k] Wmh[h,k,n]
    WS = np.einsum('qk,hkn->hqn', W, Wmh).reshape(128, N)
    WST = np.ascontiguousarray(
        WS.T.reshape(NCH, 128, 128).transpose(1, 0, 2)).astype(bf)
    bSp = (np.einsum('qk,hk->hq', W, bmh).reshape(128)
           + np.tile(bW, H)).astype(np.float32).reshape(128, 1)

    WSm = np.einsum('qk,hkn->hqn', Wm, Wmh).reshape(128, N)
    WSmT = np.ascontiguousarray(
        WSm.T.reshape(NCH, 128, 128).transpose(1, 0, 2)).astype(bf)
    bSm = (np.einsum('qk,hk->hq', Wm, bmh).reshape(128)
           + np.tile(bWm, H)).astype(np.float32).reshape(128, 1)

    whD = np.zeros((K, H), dtype=np.float32)
    for h in range(H):
        whD[h * K2:(h + 1) * K2, h] = Wh
    whD = whD.astype(bf)
    bmhT = np.ascontiguousarray(bmh.T)                            # (K, H)

    in_maps = []
    for c in range(NCORES):
        sl = slice(c * BL, (c + 1) * BL)
        in_maps.append({
            "hypT": np.ascontiguousarray(hypT_all[sl]),
            "hypN": np.ascontiguousarray(hypN_all[sl]),
            "wmhT": wmhT, "WST": WST, "WSmT": WSmT, "whD": whD,
            "bSp": bSp, "bSm": bSm, "bmhT": bmhT,
        })
    return in_maps


def kernel(hyp, Wmh, bmh, W, bW, Wm, bWm, Wh, bWh,
           dan_hidden_size=None, attention_hidden_size=None,
           multihead_size=None, **_):
    from concourse.bass_utils import run_bass_kernel_spmd

    in_maps = _prep_inputs(hyp, Wmh, bmh, W, bW, Wm, bWm, Wh, bWh)
    if "nc" not in _cache:
        _cache["nc"] = _build_nc()
    res = run_bass_kernel_spmd(_cache["nc"], in_maps, core_ids=list(range(NCORES)))
    # out is (BL, K, H) per core -> (B, H, K) -> (B, N)
    out = np.concatenate([r["out"].transpose(0, 2, 1).reshape(BL, N)
                          for r in res.results], axis=0)
    return out.astype(np.float32)


# revision 26
# speedup vs baseline: 1.0962x; 1.0008x over previous
"""Trainium2 Bass kernel for nn_Attention_46454366273781 (sparse_attention).

Reference computation (T=2048, B=32, N=1024, H=8, K=128, K2=16):
    X = einsum('tbn,hkn->bthk', hyp, Wmh) + bmh          # per-head projections
    m = X.mean(axis=1)                                   # mean over time
    g = tanh(X @ W.T + bW) * tanh(m @ Wm.T + bWm)[:,None]
    s = g @ Wh + bWh ; a = softmax(s, axis=time)
    c = einsum('bth,bthk->bhk', a, X) ; out = c.reshape(B, H*K)

Key algebra: X itself is never needed on device.
  * scoring:  X @ W.T + bW  =  hyp @ WS.T + bSp   with WS = W @ Wmh (per head)
  * gate:     m @ Wm.T + bWm = mean_t(hyp) @ WSm.T + bSm,  WSm = Wm @ Wmh
  * output:   softmax weights sum to 1, so
              c_bh = (a_bh^T hyp_b) @ Wmh_h^T + bmh_h  (the v-trick)
This turns the 137-GFLOP projection into a rank-128 scoring matmul plus two
passes over hyp (one N-major for scoring/mean, one T-major for the weighted
sum), making the kernel DMA-bound.  Sharding: data-parallel over batch B
across 8 cores (4 batches/core).  bWh cancels inside the softmax.
"""

import numpy as np
import ml_dtypes

T, B, N, H = 2048, 32, 1024, 8
K, K2 = 128, 16          # per-head dim, attention hidden per head
NCORES = 8
BL = B // NCORES         # batches per core
TC = 512                 # time chunk for scoring matmul free dim
NCH = N // 128           # contraction chunks over N
TCH = T // TC            # time chunks (scoring)
T128 = T // 128          # 128-sized time chunks

_cache = {}


def _build_nc():
    import concourse.mybir as mybir
    import concourse.tile as tile
    from concourse import bacc
    from concourse.masks import make_identity

    bf16 = mybir.dt.bfloat16
    f32 = mybir.dt.float32
    AF = mybir.ActivationFunctionType
    AX = mybir.AxisListType
    OP = mybir.AluOpType

    nc = bacc.Bacc("TRN2")
    hypT = nc.dram_tensor("hypT", (BL, N, T), bf16, kind="ExternalInput")
    hypN = nc.dram_tensor("hypN", (BL, T, N), bf16, kind="ExternalInput")
    wmhT_d = nc.dram_tensor("wmhT", (128, H, NCH, K), bf16, kind="ExternalInput")
    WST_d = nc.dram_tensor("WST", (128, NCH, 128), bf16, kind="ExternalInput")
    WSmT_d = nc.dram_tensor("WSmT", (128, NCH, 128), bf16, kind="ExternalInput")
    whD_d = nc.dram_tensor("whD", (K, H), bf16, kind="ExternalInput")
    bSp_d = nc.dram_tensor("bSp", (128, 1), f32, kind="ExternalInput")
    bSm_d = nc.dram_tensor("bSm", (128, 1), f32, kind="ExternalInput")
    bmhT_d = nc.dram_tensor("bmhT", (K, H), f32, kind="ExternalInput")
    out_d = nc.dram_tensor("out", (BL, K, H), f32, kind="ExternalOutput")

    with tile.TileContext(nc) as tc, \
         tc.tile_pool(name="wpool", bufs=1) as wpool, \
         tc.tile_pool(name="hypTp", bufs=2 * (NCH // 2)) as hypTp, \
         tc.tile_pool(name="hypNp", bufs=2 * T128) as hypNp, \
         tc.tile_pool(name="gp", bufs=4) as gp, \
         tc.tile_pool(name="aTp", bufs=2 * T128) as aTp, \
         tc.tile_pool(name="seqp", bufs=2) as seqp, \
         tc.tile_pool(name="smallp", bufs=2) as smallp, \
         tc.tile_pool(name="psA", bufs=2, space="PSUM") as psA, \
         tc.tile_pool(name="psV", bufs=2, space="PSUM") as psV, \
         tc.tile_pool(name="psS", bufs=4, space="PSUM") as psS:

        # ---- constants / weights (loaded once; wmhT last, c-phase only) ----
        WST = wpool.tile([128, NCH, 128], bf16)
        nc.gpsimd.dma_start(out=WST, in_=WST_d[:])
        WSmT = wpool.tile([128, NCH, 128], bf16)
        nc.gpsimd.dma_start(out=WSmT, in_=WSmT_d[:])
        whD = wpool.tile([K, H], bf16)
        nc.gpsimd.dma_start(out=whD, in_=whD_d[:])
        bSp = wpool.tile([128, 1], f32)
        nc.gpsimd.dma_start(out=bSp, in_=bSp_d[:])
        bSm = wpool.tile([128, 1], f32)
        nc.gpsimd.dma_start(out=bSm, in_=bSm_d[:])
        bmhT = wpool.tile([K, H], f32)
        nc.gpsimd.dma_start(out=bmhT, in_=bmhT_d[:])
        ident = wpool.tile([128, 128], bf16)
        make_identity(nc, ident)
        wmhT = wpool.tile([128, H, NCH, K], bf16)
        nc.gpsimd.dma_start(out=wmhT, in_=wmhT_d[:])
        dump = wpool.tile([128, T], bf16)   # write-only sink for mean pass

        for bl in range(BL):
            # ---- load hyp in both layouts ----
            hTg = [hypTp.tile([128, 2, T], bf16, tag="hT",
                           name=f"hT_{bl}_{i}") for i in range(NCH // 2)]
            for i in range(NCH // 2):
                nc.sync.dma_start(
                    out=hTg[i],
                    in_=hypT[bl, i * 2 * 128:(i + 1) * 2 * 128, :]
                    .rearrange("(j p) t -> p j t", p=128))
            hT = [hTg[n // 2][:, n % 2, :] for n in range(NCH)]
            hN = [hypNp.tile([128, N], bf16, tag="hN", name=f"hN_{bl}_{i}")
                  for i in range(T128)]
            for t in range(T128):
                nc.sync.dma_start(out=hN[t],
                                  in_=hypN[bl, t * 128:(t + 1) * 128, :])

            # ---- gate: mw = tanh(WSm @ mean_t(hyp)^T + bSm), packed [hq,1] ----
            hmT = smallp.tile([128, NCH], f32, tag="hmT", name=f"hmT_{bl}")
            for n in range(NCH):
                if n % 2 == 0:
                    nc.scalar.activation(out=dump, in_=hT[n], func=AF.Copy,
                                         accum_out=hmT[:, n:n + 1])
                else:
                    nc.vector.reduce_sum(out=hmT[:, n:n + 1], in_=hT[n],
                                         axis=AX.X)
            hmT_bf = smallp.tile([128, NCH], bf16, tag="hmT_bf",
                                 name=f"hmT_bf_{bl}")
            nc.scalar.activation(out=hmT_bf, in_=hmT, func=AF.Copy,
                                 scale=1.0 / T)
            ps_mwp = psS.tile([128, 1], f32, tag="psS", name=f"ps_mwp_{bl}")
            for n in range(NCH):
                nc.tensor.matmul(ps_mwp, lhsT=WSmT[:, n, :],
                                 rhs=hmT_bf[:, n:n + 1],
                                 start=(n == 0), stop=(n == NCH - 1))
            mwP = smallp.tile([128, 1], f32, tag="mwP", name=f"mwP_{bl}")
            nc.scalar.activation(out=mwP, in_=ps_mwp, func=AF.Tanh, bias=bSm)

            # ---- scoring: s = whD^T (tanh(WS hyp^T + bSp) * mwP); softmax ----
            s_exp = seqp.tile([8, T], bf16, tag="s_exp", name=f"s_exp_{bl}")
            ssum_parts = smallp.tile([8, TCH], f32, tag="ssum_parts",
                                     name=f"ssum_parts_{bl}")
            for tci in range(TCH):
                tsl = slice(tci * TC, (tci + 1) * TC)
                ps = psA.tile([128, TC], f32, tag="psA", name=f"psA_{bl}_{tci}")
                for n in range(NCH):
                    nc.tensor.matmul(ps, lhsT=WST[:, n, :],
                                     rhs=hT[n][:, tsl],
                                     start=(n == 0), stop=(n == NCH - 1))
                g1 = gp.tile([128, TC], f32, tag="g1", name=f"g1_{bl}_{tci}")
                nc.scalar.activation(out=g1, in_=ps, func=AF.Tanh, bias=bSp)
                g2 = gp.tile([128, TC], bf16, tag="g2", name=f"g2_{bl}_{tci}")
                nc.vector.tensor_scalar_mul(g2, g1, mwP)
                ps_s = psS.tile([8, TC], f32, tag="psS", name=f"ps_s_{bl}_{tci}")
                nc.tensor.matmul(ps_s, lhsT=whD, rhs=g2, start=True, stop=True)
                nc.scalar.activation(out=s_exp[:, tsl], in_=ps_s, func=AF.Exp,
                                     accum_out=ssum_parts[:, tci:tci + 1])
            ssum = smallp.tile([8, 1], f32, tag="ssum", name=f"ssum_{bl}")
            nc.vector.reduce_sum(out=ssum, in_=ssum_parts, axis=AX.X)
            sinv = smallp.tile([8, 1], f32, tag="sinv", name=f"sinv_{bl}")
            nc.vector.reciprocal(sinv, ssum)

            # ---- v = a^T hyp ; c^T = v @ Wmh^T + bmh ----
            aT = []
            for t in range(T128):
                ps_aT = psS.tile([128, 8], bf16, tag="psS",
                                 name=f"ps_aT_{bl}_{t}")
                nc.tensor.transpose(ps_aT, s_exp[:, t * 128:(t + 1) * 128],
                                    ident[:8, :8])
                aTt = aTp.tile([128, 8], bf16, tag="aT", name=f"aT_{bl}_{t}")
                nc.scalar.copy(aTt, ps_aT)
                aT.append(aTt)
            v_sb = smallp.tile([8, N], bf16, tag="v_sb", name=f"v_sb_{bl}")
            for nh in range(2):
                ps_v = psV.tile([8, 512], f32, tag="psV",
                                name=f"ps_v_{bl}_{nh}")
                for t in range(T128):
                    nc.tensor.matmul(ps_v, lhsT=aT[t],
                                     rhs=hN[t][:, nh * 512:(nh + 1) * 512],
                                     start=(t == 0), stop=(t == T128 - 1))
                nc.scalar.activation(out=v_sb[:, nh * 512:(nh + 1) * 512],
                                     in_=ps_v, func=AF.Copy, scale=sinv)
            vT = smallp.tile([128, NCH, 8], bf16, tag="vT", name=f"vT_{bl}")
            for n in range(NCH):
                ps_vT = psS.tile([128, 8], bf16, tag="psS",
                                 name=f"ps_vT_{bl}_{n}")
                nc.tensor.transpose(ps_vT, v_sb[:, n * 128:(n + 1) * 128],
                                    ident[:8, :8])
                nc.scalar.copy(vT[:, n, :], ps_vT)
            ps_cT = psS.tile([128, H], f32, tag="psS", name=f"ps_cT_{bl}")
            for h in range(H):
                for n in range(NCH):
                    nc.tensor.matmul(ps_cT[:, h:h + 1], lhsT=wmhT[:, h, n, :],
                                     rhs=vT[:, n, h:h + 1],
                                     start=(n == 0), stop=(n == NCH - 1),
                                     skip_group_check=True)
            c2 = smallp.tile([128, H], f32, tag="c2", name=f"c2_{bl}")
            nc.vector.tensor_tensor(out=c2, in0=ps_cT, in1=bmhT, op=OP.add)
            nc.sync.dma_start(out=out_d[bl], in_=c2)

    nc.compile()
    return nc


def _prep_inputs(hyp, Wmh, bmh, W, bW, Wm, bWm, Wh, bWh):
    """Host-side sharding + layout prep (numpy only)."""
    bf = ml_dtypes.bfloat16
    hyp = np.asarray(hyp, np.float32)
    Wmh = np.asarray(Wmh, np.float32)
    bmh = np.asarray(bmh, np.float32)
    W = np.asarray(W, np.float32)
    bW = np.asarray(bW, np.float32)
    Wm = np.asarray(Wm, np.float32)
    bWm = np.asarray(bWm, np.float32)
    Wh = np.asarray(Wh, np.float32)

    hyp_b = np.ascontiguousarray(hyp.transpose(1, 0, 2))          # (B, T, N)
    hypN_all = hyp_b.astype(bf)
    hypT_all = np.ascontiguousarray(hyp_b.transpose(0, 2, 1)).astype(bf)

    # Wmh (H, K, N) -> (128, H, NCH, K): [p, h, n, k] = Wmh[h, k, n*128+p]
    wmhT = np.ascontiguousarray(
        Wmh.transpose(2, 0, 1).reshape(NCH, 128, H, K)
        .transpose(1, 2, 0, 3)).astype(bf)

    # fused scoring weights: WS[h*16+q, n] = sum_k W[q,k] Wmh[h,k,n]
    WS = np.einsum('qk,hkn->hqn', W, Wmh).reshape(128, N)
    WST = np.ascontiguousarray(
        WS.T.reshape(NCH, 128, 128).transpose(1, 0, 2)).astype(bf)
    bSp = (np.einsum('qk,hk->hq', W, bmh).reshape(128)
           + np.tile(bW, H)).astype(np.float32).reshape(128, 1)

    WSm = np.einsum('qk,hkn->hqn', Wm, Wmh).reshape(128, N)
    WSmT = np.ascontiguousarray(
        WSm.T.reshape(NCH, 128, 128).transpose(1, 0, 2)).astype(bf)
    bSm = (np.einsum('qk,hk->hq', Wm, bmh).reshape(128)
           + np.tile(bWm, H)).astype(np.float32).reshape(128, 1)

    whD = np.zeros((K, H), dtype=np.float32)
    for h in range(H):
        whD[h * K2:(h + 1) * K2, h] = Wh
    whD = whD.astype(bf)
    bmhT = np.ascontiguousarray(bmh.T)                            # (K, H)

    in_maps = []
    for c in range(NCORES):
        sl = slice(c * BL, (c + 1) * BL)
        in_maps.append({
            "hypT": np.ascontiguousarray(hypT_all[sl]),
            "hypN": np.ascontiguousarray(hypN_all[sl]),
            "wmhT": wmhT, "WST": WST, "WSmT": WSmT, "whD": whD,
            "bSp": bSp, "bSm": bSm, "bmhT": bmhT,
        })
    return in_maps


def kernel(hyp, Wmh, bmh, W, bW, Wm, bWm, Wh, bWh,
           dan_hidden_size=None, attention_hidden_size=None,
           multihead_size=None, **_):
    from concourse.bass_utils import run_bass_kernel_spmd

    in_maps = _prep_inputs(hyp, Wmh, bmh, W, bW, Wm, bWm, Wh, bWh)
    if "nc" not in _cache:
        _cache["nc"] = _build_nc()
    res = run_bass_kernel_spmd(_cache["nc"], in_maps, core_ids=list(range(NCORES)))
    # out is (BL, K, H) per core -> (B, H, K) -> (B, N)
    out = np.concatenate([r["out"].transpose(0, 2, 1).reshape(BL, N)
                          for r in res.results], axis=0)
    return out.astype(np.float32)
